# revision 1
# baseline (speedup 1.0000x reference)
"""Trainium2 Bass kernel for nn_MultiHeadSelfAttention2d.

Reference computation (B=1, C=64, H=32, W=128, HEADS=8, HIDDEN=16):
  q/k/v = 1x1 conv over channels (+bias), per-head attention over N=H*W=4096
  positions, softmax(q k^T / sqrt(16)), out = attn @ v, then a Linear over the
  W axis (W == HEADS*HIDDEN == 128) producing (1, 128, 32, 64).

Distribution: one (batch, head) pair per NeuronCore -> 8 cores, fully
independent (no collectives).  Each core computes its head's 16 output
channels of the final Linear; the host concatenates.

Per-core dataflow (all on one NeuronCore):
  - proj:   Q,K = W x + b via PE matmul (stationary has Q/K replicated into
            partition quadrants 0-15 / 32-47 for row-tiled QK matmuls)
  - V^T:    computed directly in [m, d] layout via X-stationary matmuls with
            an appended ones-row/ones-col -> V_aug [m, 17] (col 16 == 1.0,
            which makes the AV matmul also produce the softmax denominator)
  - S^T:    S^T[m,n] = K^T Q as 32x128-mode row-tiled matmuls (contraction
            dim is only 16), two m-chunks concurrently in the PE array
  - exp:    exp(S/4) from PSUM -> SBUF bf16, split between the Scalar engine
            (hardware exp) and the Vector engine (custom cubic-poly exp op;
            logits are in [-0.25, 0.25] so the poly is exact to ~1e-4 rel)
  - AV:     O_un^T[d,n] (+ rowsum row) = V_aug^T @ P^T accumulated over m in
            PSUM, 4 column-tiles (128x32 mode) concurrently
  - norm:   PE-transpose [17,128] blocks -> [128,17], reciprocal of rowsum,
            per-partition scale on the copy back
  - linear: out[(h,c), o] = O_fin^T @ w_lin^T + b_lin, DMA out [512, 64]
"""

import os
from contextlib import ExitStack

import ml_dtypes
import numpy as np

import concourse.bass as bass
import concourse.tile as tile
from concourse import bacc, mybir

# ---------------------------------------------------------------------------
# Problem constants (hardcoded per the task contract)
HEADS = 8
HID = 16
C_IN = 64
OUT_DIM = 64
H_IMG = 32
W_IMG = 128
N_TOK = H_IMG * W_IMG  # 4096
N_CORES = 8

BF16 = mybir.dt.bfloat16
F32 = mybir.dt.float32

# ---------------------------------------------------------------------------
# Custom DVE (vector engine) op: out = (((c3*u + c2)*u + c1)*u + 1)^2
# With c1=1/8, c2=1/128, c3=1/3072 this is exp(u/4) to ~1e-4 rel for |u|<1.3
# (|S| observed < 0.85).  Lets the Vector engine share softmax-exp work with
# the Scalar engine, which is otherwise the kernel bottleneck.
from concourse.dve_spec import Spec, Src0, C0, C1, C2, One, sq, lower
from concourse.dve_uop import DveOpSpec
from concourse import dve_ops
from concourse.dve_table_gen import dve_ver_for

EXP_C1 = 1.0 / 8.0
EXP_C2 = 1.0 / 128.0
EXP_C3 = 1.0 / 3072.0


def _exp_ref(in0, in1, c0, c1, c2):
    u = in0.astype(np.float32)
    q = ((np.float32(c2) * u + np.float32(c1)) * u + np.float32(c0)) * u + np.float32(
        1.0
    )
    return q * q


def _register_exp_op():
    name = "EXP_QTR_POLY_ANT"
    for op in dve_ops.OPS:
        if op.name == name:
            return op
    body = sq(((Src0 * C2 + C1) * Src0 + C0) * Src0 + One)
    spec = Spec(body=body, reference=_exp_ref)
    # Register the opcode row first, then compute the uop sha for each ver so
    # DveOp.compile()'s drift check passes.
    row = max(dve_ops._SUB_OPCODE_FOR_NAME.values()) + 1
    assert row < 0x20
    dve_ops._SUB_OPCODE_FOR_NAME[name] = row
    shas = {}
    for ver in ("v3", "v4"):
        try:
            uops = lower(spec, ver=ver)
            shas[ver] = DveOpSpec(name=name, opcode=row, uops=uops, rd1_en=False).sha(
                ver
            )
        except Exception:
            pass
    op = dve_ops.DveOp(name, spec, subdim=False, uops_sha=shas)
    dve_ops.OPS.append(op)
    dve_ops.CUSTOM_DVE_SPECS[name] = spec
    return op


EXP_OP = _register_exp_op()


# ---------------------------------------------------------------------------
def build_module(
    n_tok: int = N_TOK,
    act_exp_per_8: int = 7,
    av_flush: int = 4,
    s_bufs: int = 3,
    pt_bufs: int = 6,
    av_bufs: int = 1,
    misc_bufs: int = 4,
    exp_w: int = 1024,
    av_diag: bool = False,
    reps: int = 1,
    skip_av: bool = False,
    av_iso: int = 0,
    skip_tail: bool = False,
    skip_attn: bool = False,
):
    """Builds (and bacc-compiles) the per-core Bass module.

    n_tok: number of token positions (4096 full size; smaller for sim tests;
           must be a multiple of 1024 ... for 512 we special-case NB).
    act_exp_per_8: out of every 8 exp instructions, how many go to the Scalar
           engine (rest go to the Vector engine custom poly op).
    """
    n_blk = min(2048, n_tok)  # n block per AV accumulator sweep
    assert n_tok % n_blk == 0
    exp_w = min(exp_w, n_blk)  # exp instruction width
    n_halves = n_blk // exp_w  # exp instructions per (nb, m-chunk)
    mch = n_tok // 128  # number of m chunks
    n_hrows = n_tok // W_IMG  # h rows (32 full size)
    f_tot = n_hrows * HID  # (h, c) rows of the final output
    assert f_tot % 64 == 0

    nc = bacc.Bacc()

    xin = nc.dram_tensor("xin", [C_IN + 1, n_tok], BF16, kind="ExternalInput")
    wq2 = nc.dram_tensor("wq2", [C_IN, 128], BF16, kind="ExternalInput")
    wk2 = nc.dram_tensor("wk2", [C_IN, 128], BF16, kind="ExternalInput")
    bq2 = nc.dram_tensor("bq2", [128, 1], F32, kind="ExternalInput")
    bk2 = nc.dram_tensor("bk2", [128, 1], F32, kind="ExternalInput")
    wva = nc.dram_tensor("wva", [C_IN + 1, HID + 1], BF16, kind="ExternalInput")
    wlt = nc.dram_tensor("wlt", [W_IMG, OUT_DIM], BF16, kind="ExternalInput")
    blb = nc.dram_tensor("blb", [128, OUT_DIM], F32, kind="ExternalInput")
    idt = nc.dram_tensor("idt", [32, 32], BF16, kind="ExternalInput")
    out = nc.dram_tensor("out", [f_tot, OUT_DIM], F32, kind="ExternalOutput")

    # exp engine schedule (ACT vs DVE), round-robin at act_exp_per_8 / 8
    exp_counter = [0]

    # spread the DVE-assigned chunks evenly through the stream
    dve_slots = {
        0: set(),
        1: {7},
        2: {3, 7},
        3: {2, 5, 7},
        4: {1, 3, 5, 7},
    }[8 - act_exp_per_8 if act_exp_per_8 >= 4 else 4]

    def exp_on_act():
        i = exp_counter[0] % 8
        exp_counter[0] += 1
        return i not in dve_slots

    with tile.TileContext(nc) as tc, ExitStack() as ctx:
        const = ctx.enter_context(tc.tile_pool(name="const", bufs=1))
        pt_pool = ctx.enter_context(tc.tile_pool(name="pt_pool", bufs=pt_bufs))

        # ---- constant loads -------------------------------------------------
        XB = const.tile([C_IN + 1, n_tok], BF16)
        nc.sync.dma_start(XB[:], xin.ap())
        WQ2 = const.tile([C_IN, 128], BF16)
        nc.sync.dma_start(WQ2[:], wq2.ap())
        WK2 = const.tile([C_IN, 128], BF16)
        nc.sync.dma_start(WK2[:], wk2.ap())
        BQ2 = const.tile([128, 1], F32)
        nc.sync.dma_start(BQ2[:], bq2.ap())
        BK2 = const.tile([128, 1], F32)
        nc.sync.dma_start(BK2[:], bk2.ap())
        WVA = const.tile([C_IN + 1, HID + 1], BF16)
        nc.sync.dma_start(WVA[:], wva.ap())
        WL = const.tile([W_IMG, OUT_DIM], BF16)
        nc.sync.dma_start(WL[:], wlt.ap())
        BLB = const.tile([128, OUT_DIM], F32)
        nc.sync.dma_start(BLB[:], blb.ap())
        IDT = const.tile([32, 32], BF16)
        nc.sync.dma_start(IDT[:], idt.ap())

        rep_ctx = (
            tc.For_i(
                0,
                reps,
                1,
                hint_engines=(
                    mybir.EngineType.PE,
                    mybir.EngineType.Activation,
                    mybir.EngineType.DVE,
                    mybir.EngineType.SP,
                ),
            )
            if reps > 1
            else None
        )
        if rep_ctx is not None:
            rep_ctx.__enter__()

        QT = const.tile([128, n_tok], BF16)  # Q replicated in all 4 quadrants
        KT = const.tile([128, n_tok], BF16)  # K replicated in all 4 quadrants
        VA = const.tile([128, mch * (HID + 1)], BF16)  # V_aug^T per m-chunk
        OU = const.tile([HID + 1, n_tok], BF16)  # unnormalized O^T + rowsum
        OF = const.tile([128, f_tot], BF16)  # normalized O in [w, (h,c)]
        RC = const.tile([128, n_hrows], F32)  # per-(h,w) reciprocal rowsums

        # ---- phase 1+2: projections + V_aug^T (own psum pool) --------------
        with tc.tile_pool(name="p12_ps", bufs=misc_bufs, space="PSUM") as p12_ps:
            for ch in range(n_tok // 512):
                cs = slice(ch * 512, ch * 512 + 512)
                psq = p12_ps.tile([128, 512], F32, tag="m")
                nc.tensor.matmul(psq[:], lhsT=WQ2[:], rhs=XB[0:C_IN, cs])
                nc.scalar.activation(
                    QT[:, cs], psq[:], mybir.ActivationFunctionType.Identity, bias=BQ2[:]
                )
                psk = p12_ps.tile([128, 512], F32, tag="m")
                nc.tensor.matmul(psk[:], lhsT=WK2[:], rhs=XB[0:C_IN, cs])
                nc.vector.tensor_scalar_add(KT[:, cs], psk[:], BK2[:])

            for mc in range(mch):
                ms = slice(mc * 128, mc * 128 + 128)
                vs = slice(mc * (HID + 1), (mc + 1) * (HID + 1))
                psv = p12_ps.tile([128, HID + 1], F32, tag="m")
                nc.tensor.matmul(psv[:], lhsT=XB[:, ms], rhs=WVA[:])
                if mc % 2 == 0:
                    nc.scalar.copy(VA[:, vs], psv[:])
                else:
                    nc.vector.tensor_copy(VA[:, vs], psv[:])

        # ---- phase 3: attention (own psum pools) ---------------------------
        if skip_av or skip_attn:
            nc.gpsimd.memset(OU[:], 1.0)
        with tc.tile_pool(name="s_pool", bufs=s_bufs, space="PSUM") as s_pool, \
             tc.tile_pool(name="av_pool", bufs=av_bufs, space="PSUM") as av_pool:
            for nb in range(0 if skip_attn else n_tok // n_blk):
                n0 = nb * n_blk
                sub_w = n_blk // 4
                # accumulator; av_diag: col-tile group c gets its own bank
                av = av_pool.tile([128, n_blk if av_diag else 512], F32, tag="av")
                pending = []
                for mc in range(mch):
                    grp = 32 * (mc % 4)
                    pt_t = pt_pool.tile([128, n_blk], BF16, tag="pt")
                    for hh in range(n_halves):
                        s = s_pool.tile([128, exp_w], F32, tag="s")
                        for sub in range(exp_w // 512):
                            o0 = hh * exp_w + sub * 512
                            nc.tensor.matmul(
                                s[:, sub * 512 : sub * 512 + 512],
                                lhsT=KT[grp : grp + HID, mc * 128 : mc * 128 + 128],
                                rhs=QT[grp : grp + HID, n0 + o0 : n0 + o0 + 512],
                                tile_position=(grp, 0),
                            )
                        dst = pt_t[:, hh * exp_w : (hh + 1) * exp_w]
                        if exp_on_act():
                            nc.scalar.activation(
                                dst, s[:], mybir.ActivationFunctionType.Exp, scale=0.25
                            )
                        else:
                            nc.vector._custom_dve(
                                EXP_OP, out=dst, in0=s[:], s0=EXP_C1, s1=EXP_C2, imm2=EXP_C3
                            )
                    pending.append((mc, pt_t))

                    def av_burst(mcj, ptj):
                        nochain = av_iso & 1
                        for c in range(4):
                            rhs = (
                                QT[0:128, 0:sub_w]
                                if (av_iso & 2)
                                else ptj[:, c * sub_w : (c + 1) * sub_w]
                            )
                            nc.tensor.matmul(
                                av[32 * c : 32 * c + HID + 1,
                                   c * sub_w : (c + 1) * sub_w]
                                if av_diag
                                else av[32 * c : 32 * c + HID + 1, 0:sub_w],
                                lhsT=VA[:, mcj * (HID + 1) : (mcj + 1) * (HID + 1)],
                                rhs=rhs,
                                tile_position=(0, 32 * c),
                                start=True if nochain else (mcj == 0),
                                stop=True if nochain else (mcj == mch - 1),
                                skip_group_check=True,
                            )

                    if skip_av:
                        pending = []
                    elif len(pending) > av_flush:
                        # lagged emission: by the time the PE reaches this AV
                        # burst, its exp is provably complete (the s-slot the
                        # current QK chunk just claimed was freed by it).
                        av_burst(*pending.pop(0))
                if not skip_av:
                    for mcj, ptj in pending:
                        av_burst(mcj, ptj)
                    pending = []
                # flush O_un^T for this n block
                for c in range(0 if skip_av else 4):
                    dst = OU[:, n0 + c * sub_w : n0 + (c + 1) * sub_w]
                    srcv = (
                        av[32 * c : 32 * c + HID + 1, c * sub_w : (c + 1) * sub_w]
                        if av_diag
                        else av[32 * c : 32 * c + HID + 1, 0:sub_w]
                    )
                    if c % 4 != 3:
                        nc.scalar.copy(dst, srcv)
                    else:
                        nc.vector.tensor_copy(dst, srcv)

        # ---- phase 4+5: transpose + normalize + final linear ---------------
        if skip_tail:
            nc.gpsimd.memset(OF[:], 0.5)
            nc.gpsimd.memset(RC[:], 1.0)
        with tc.tile_pool(name="tail_ps", bufs=1, space="PSUM") as tail_ps:
            for hb in range(0 if skip_tail else n_hrows):
                pst = tail_ps.tile([128, HID + 1], BF16, tag="t4", bufs=6)
                nc.tensor.transpose(
                    pst[:], OU[:, hb * 128 : hb * 128 + 128], IDT[0 : HID + 1, 0 : HID + 1]
                )
                rc = pt_pool.tile([128, 1], F32, tag="rc", bufs=8)
                nc.vector.reciprocal(rc[:], pst[:, HID : HID + 1])
                fs = slice(hb * HID, (hb + 1) * HID)
                if hb % 2 == 0:
                    nc.scalar.activation(
                        OF[:, fs],
                        pst[:, 0:HID],
                        mybir.ActivationFunctionType.Copy,
                        scale=rc[:],
                    )
                else:
                    nc.vector.tensor_scalar_mul(OF[:, fs], pst[:, 0:HID], rc[:])

            for qi in range((f_tot + 127) // 128):
                fw = min(128, f_tot - qi * 128)
                fs = slice(qi * 128, qi * 128 + fw)
                psf = tail_ps.tile([128, OUT_DIM], F32, tag="fin", bufs=2)
                nc.tensor.matmul(psf[0:fw, :], lhsT=OF[:, fs], rhs=WL[:])
                res = pt_pool.tile([128, OUT_DIM], F32, tag="res", bufs=2)
                nc.vector.tensor_add(res[0:fw, :], psf[0:fw, :], BLB[0:fw, :])
                nc.sync.dma_start(out.ap()[fs, :], res[0:fw, :])

        if rep_ctx is not None:
            rep_ctx.__exit__(None, None, None)

    nc.compile()
    return nc


# ---------------------------------------------------------------------------
def make_core_inputs(x, wq, bq, wk, bk, wv, bv, w_lin, b_lin, n_tok=N_TOK):
    """Host-side prep: full inputs -> list of 8 per-core input dicts."""
    X = np.asarray(x, np.float32).reshape(C_IN, -1)[:, :n_tok]
    xa = np.ones((C_IN + 1, n_tok), np.float32)
    xa[:C_IN] = X
    xin = xa.astype(ml_dtypes.bfloat16)
    wlt = np.ascontiguousarray(np.asarray(w_lin, np.float32).T).astype(
        ml_dtypes.bfloat16
    )
    blb = np.tile(np.asarray(b_lin, np.float32)[None, :], (128, 1)).astype(np.float32)
    idt = np.eye(32, dtype=np.float32).astype(ml_dtypes.bfloat16)

    maps = []
    for h in range(HEADS):
        sl = slice(HID * h, HID * (h + 1))
        wq_h = np.asarray(wq, np.float32)[sl]
        wk_h = np.asarray(wk, np.float32)[sl]
        wv_h = np.asarray(wv, np.float32)[sl]
        w2 = np.zeros((C_IN, 128), np.float32)
        k2 = np.zeros((C_IN, 128), np.float32)
        b2 = np.zeros((128, 1), np.float32)
        bk2_ = np.zeros((128, 1), np.float32)
        for qd in range(4):
            w2[:, 32 * qd : 32 * qd + HID] = wq_h.T
            k2[:, 32 * qd : 32 * qd + HID] = wk_h.T
            b2[32 * qd : 32 * qd + HID, 0] = np.asarray(bq, np.float32)[sl]
            bk2_[32 * qd : 32 * qd + HID, 0] = np.asarray(bk, np.float32)[sl]
        wva_ = np.zeros((C_IN + 1, HID + 1), np.float32)
        wva_[0:C_IN, 0:HID] = wv_h.T
        wva_[C_IN, 0:HID] = np.asarray(bv, np.float32)[sl]
        wva_[C_IN, HID] = 1.0
        maps.append(
            {
                "xin": xin,
                "wq2": w2.astype(ml_dtypes.bfloat16),
                "wk2": k2.astype(ml_dtypes.bfloat16),
                "bq2": b2,
                "bk2": bk2_,
                "wva": wva_.astype(ml_dtypes.bfloat16),
                "wlt": wlt,
                "blb": blb,
                "idt": idt,
            }
        )
    return maps


_MODULE_CACHE = {}


def _get_module(**kw):
    key = tuple(sorted(kw.items()))
    if key not in _MODULE_CACHE:
        _MODULE_CACHE[key] = build_module(**kw)
    return _MODULE_CACHE[key]


def kernel(x, wq, bq, wk, bk, wv, bv, w_lin, b_lin):
    from concourse.bass_utils import run_bass_kernel_spmd

    nc = _get_module()
    in_maps = make_core_inputs(x, wq, bq, wk, bk, wv, bv, w_lin, b_lin)
    res = run_bass_kernel_spmd(nc, in_maps, core_ids=list(range(N_CORES)))
    full = np.empty((1, HEADS * HID, H_IMG, OUT_DIM), np.float32)
    for h in range(HEADS):
        o = res.results[h]["out"].reshape(H_IMG, HID, OUT_DIM)
        full[0, HID * h : HID * (h + 1)] = o.transpose(1, 0, 2)
    return full



# revision 35
# speedup vs baseline: 1.5838x; 1.5838x over previous
"""Trainium2 Bass kernel for nn_MultiHeadSelfAttention2d (fp8 redesign).

Reference computation (B=1, C=64, H=32, W=128, HEADS=8, HIDDEN=16):
  q/k/v = 1x1 conv over channels (+bias), per-head attention over N=H*W=4096
  positions, softmax(q k^T / sqrt(16)), out = attn @ v, then a Linear over the
  W axis (W == HEADS*HIDDEN == 128) producing (1, 128, 32, 64).

Distribution: one head per NeuronCore -> 8 cores, fully independent.

Per-core dataflow:
  - proj:   V^T [m, d_aug] via X-stationary bf16 matmuls (bias + ones col
            folded into the augmented wva, all scaled x16), converted to
            fp8e4.  Q,K = W_aug x (bias row folded, x16) -> PSUM -> fp8e4
            in [d=16, (q|k), plane, n] layout where plane 1 is zeros.
  - S^T:    fp8 DoubleRow matmuls: lhsT = K[16,2,128] (plane1 = 0), rhs =
            Q[16,2,512] -> S^T*256 in PSUM at 0.5 cycles/col.
  - exp:    exp(u/1024) from PSUM -> fp8e4 P^T pair tiles [128, 2, n],
            split between ACT (hw exp) and DVE (cubic-poly custom op) by a
            greedy static schedule.
  - AV:     transposed accumulation: stationary = P^T pair [128,2,128],
            moving = V_aug pair [128,2,17] -> out [128(n=w), 17] per image
            row h, PSUM-accumulated over the 16 m-chunk pairs (DoubleRow).
            Output arrives already transposed; col 16 is the softmax
            denominator (V_aug ones column).
  - norm:   strided reciprocal of col 16, stride-0-broadcast tensor_tensor
            multiply -> OF [128(w), 32(h), 16(c)] bf16.
  - linear: out[o, (h,c)] = W_lin^T-stationary matmul over w + per-partition
            bias -> DMA out [64, 512] f32 (host transposes).
"""

import os
from contextlib import ExitStack

import ml_dtypes
import numpy as np

import concourse.bass as bass
import concourse.tile as tile
from concourse import bacc, mybir

# ---------------------------------------------------------------------------
# Problem constants (hardcoded per the task contract)
HEADS = 8
HID = 16
C_IN = 64
OUT_DIM = 64
H_IMG = 32
W_IMG = 128
N_TOK = H_IMG * W_IMG  # 4096
N_CORES = 8

BF16 = mybir.dt.bfloat16
F32 = mybir.dt.float32
FP8 = mybir.dt.float8e4
NPF8 = ml_dtypes.float8_e4m3

QK_SCALE = 16.0  # host-side scale on wq/wk/wv (and their biases)

# ---------------------------------------------------------------------------
# Custom DVE (vector engine) op: out = (((c3*u + c2)*u + c1)*u + 1)^2
# With c1=1/2048, c2=1/(2*2048^2), c3=1/(6*2048^3) this is exp(u/1024) to
# ~1e-5 rel for |u| < 220 (S*256 observed < 220).  Lets the Vector engine
# share softmax-exp work with the Scalar engine.
from concourse.dve_spec import Spec, Src0, C0, C1, C2, One, sq, lower
from concourse.dve_uop import DveOpSpec
from concourse import dve_ops
from concourse.dve_table_gen import dve_ver_for

EXP_C1 = 1.0 / 2048.0
EXP_C2 = 1.0 / (2.0 * 2048.0**2)
EXP_C3 = 1.0 / (6.0 * 2048.0**3)


def _exp_ref(in0, in1, c0, c1, c2):
    u = in0.astype(np.float32)
    q = ((np.float32(c2) * u + np.float32(c1)) * u + np.float32(c0)) * u + np.float32(
        1.0
    )
    return q * q


def _register_exp_op():
    name = "EXP_QTR_POLY_ANT"
    for op in dve_ops.OPS:
        if op.name == name:
            return op
    body = sq(((Src0 * C2 + C1) * Src0 + C0) * Src0 + One)
    spec = Spec(body=body, reference=_exp_ref)
    row = max(dve_ops._SUB_OPCODE_FOR_NAME.values()) + 1
    assert row < 0x20
    dve_ops._SUB_OPCODE_FOR_NAME[name] = row
    shas = {}
    for ver in ("v3", "v4"):
        try:
            uops = lower(spec, ver=ver)
            shas[ver] = DveOpSpec(name=name, opcode=row, uops=uops, rd1_en=False).sha(
                ver
            )
        except Exception:
            pass
    op = dve_ops.DveOp(name, spec, subdim=False, uops_sha=shas)
    dve_ops.OPS.append(op)
    dve_ops.CUSTOM_DVE_SPECS[name] = spec
    return op


EXP_OP = _register_exp_op()

# expm1 variant: out = ((c2*u + c1)*u + c0)*u  ~=  exp(u/1024) - 1
# (delta form keeps fp8 quantization error ~4x smaller near P=1)
EXPM1_C0 = 1.0 / 1024.0
EXPM1_C1 = 1.0 / 2097152.0
EXPM1_C2 = 4.0 / (3.0 * 2048.0**3)


def _expm1_ref(in0, in1, c0, c1, c2):
    u = in0.astype(np.float32)
    return ((np.float32(c2) * u + np.float32(c1)) * u + np.float32(c0)) * u


def _register_expm1_op():
    name = "EXPM1_CUBIC_ANT"
    for op in dve_ops.OPS:
        if op.name == name:
            return op
    body = ((Src0 * C2 + C1) * Src0 + C0) * Src0
    spec = Spec(body=body, reference=_expm1_ref)
    row = max(dve_ops._SUB_OPCODE_FOR_NAME.values()) + 1
    assert row < 0x20
    dve_ops._SUB_OPCODE_FOR_NAME[name] = row
    shas = {}
    for ver in ("v3", "v4"):
        try:
            uops = lower(spec, ver=ver)
            shas[ver] = DveOpSpec(name=name, opcode=row, uops=uops, rd1_en=False).sha(
                ver
            )
        except Exception:
            pass
    op = dve_ops.DveOp(name, spec, subdim=False, uops_sha=shas)
    dve_ops.OPS.append(op)
    dve_ops.CUSTOM_DVE_SPECS[name] = spec
    return op


EXPM1_OP = _register_expm1_op()

# m-chunks whose exp runs on DVE as delta=expm1 (odd chunks minus {1,3}):
# 14 of 32, balancing ACT 18*4*1038ns vs DVE 14*8*658ns.
D_CHUNKS = frozenset(mc for mc in range(1, 32, 2)) - {1, 3}


def _chunk_splits(n):
    """Split n columns into s-tile widths of 1024 (PSUM: 2 banks/tile)."""
    assert n % 1024 == 0
    return [(o, 1024) for o in range(0, n, 1024)]


# ---------------------------------------------------------------------------
def build_module(n_tok: int = N_TOK, act_bias_ns: float = 0.0, pt_bufs: int = 4):
    """Builds (and bacc-compiles) the per-core Bass module."""
    assert n_tok % 1024 == 0
    mch = n_tok // 128  # m chunks (32)
    npair = mch // 2  # DoubleRow pairs (16)
    nh = n_tok // W_IMG  # image rows (32)
    f_tot = nh * HID  # (h,c) columns of the final output (512)

    nc = bacc.Bacc()

    xin = nc.dram_tensor("xin", [C_IN + 1, n_tok], BF16, kind="ExternalInput")
    wpk = nc.dram_tensor("wpk", [C_IN + 1, 2 * HID + HID + 1], BF16, kind="ExternalInput")
    ztil = nc.dram_tensor("ztil_v3", [HID, n_tok], FP8, kind="ExternalInput")
    wlt = nc.dram_tensor("wlt", [W_IMG, OUT_DIM], BF16, kind="ExternalInput")
    blb = nc.dram_tensor("blb", [OUT_DIM, 1], F32, kind="ExternalInput")
    out = nc.dram_tensor("out", [OUT_DIM, f_tot], F32, kind="ExternalOutput")

    # --- engine assignment: time-balanced greedy over per-engine rings -----
    ACT_RATE, ACT_INIT = 1.0 / 1.2, 185.0
    DVE_RATE, DVE_INIT = 1.0 / 0.96, 125.0
    eng_t = {"A": 1283.0 + act_bias_ns, "D": 0.0}

    def charge_engine(eng, width):
        if eng == "A":
            eng_t["A"] += width * ACT_RATE + ACT_INIT
        else:
            eng_t["D"] += width * DVE_RATE + DVE_INIT

    def pick_engine(width_a, width_d=None):
        """Pick engine for an op of width_a on ACT / width_d on DVE."""
        if width_d is None:
            width_d = width_a
        ca = width_a * ACT_RATE + ACT_INIT
        cd = width_d * DVE_RATE + DVE_INIT
        if eng_t["A"] + ca <= eng_t["D"] + cd:
            eng_t["A"] += ca
            return "A"
        eng_t["D"] += cd
        return "D"

    with tile.TileContext(nc) as tc, ExitStack() as ctx:
        const = ctx.enter_context(tc.tile_pool(name="const", bufs=1))

        WPK = const.tile([C_IN + 1, 2 * HID + HID + 1], BF16)
        nc.sync.dma_start(WPK[:], wpk.ap())
        WQA = WPK[:, 0:HID]
        WKA = WPK[:, HID : 2 * HID]
        WVA = WPK[:, 2 * HID : 3 * HID + 1]

        XB = const.tile([C_IN + 1, n_tok], BF16)
        QK8 = const.tile([HID, 2, 2, n_tok], FP8)
        WL = const.tile([W_IMG, OUT_DIM], BF16)
        BLB = const.tile([OUT_DIM, 1], F32)
        for q in range(4):
            qs = slice(q * (n_tok // 4), (q + 1) * (n_tok // 4))
            nc.sync.dma_start(XB[:, qs], xin.ap()[:, qs])
        nc.sync.dma_start(QK8[:, 0, 1, :], ztil.ap())
        nc.sync.dma_start(QK8[:, 1, 1, :], ztil.ap())
        nc.sync.dma_start(WL[:], wlt.ap())
        nc.sync.dma_start(BLB[:], blb.ap())

        VA = const.tile([128, npair, 2, HID + 1], FP8)
        VASD = const.tile([128, 4, HID + 1], F32)  # per-group D-chunk V sums
        VSD = const.tile([128, HID + 1], F32)  # sum over all D-chunk m of V_aug
        VSDH = const.tile([128, HID + 1], BF16)
        VSDL = const.tile([128, HID + 1], BF16)
        ONESB = const.tile([128, 128], BF16)
        RC = const.tile([128, nh], F32)
        OF = const.tile([128, nh, HID], BF16)
        RES = const.tile([OUT_DIM, f_tot], F32)

        # ---- interleaved Q/K + V projection, fp8 convert ------------------
        with tc.tile_pool(name="pv", bufs=2, space="PSUM") as pv, \
             tc.tile_pool(name="pqk", bufs=2, space="PSUM") as pqk:
            for sc in range(n_tok // 512):
                ps = pqk.tile([HID, 1024], F32, tag="qk")
                cs = slice(512 * sc, 512 * sc + 512)
                nc.tensor.matmul(ps[:, 0:512], lhsT=WQA, rhs=XB[:, cs])
                nc.tensor.matmul(ps[:, 512:1024], lhsT=WKA, rhs=XB[:, cs])
                src_ap = ps[:].rearrange("p (a b) -> p a b", b=512)
                dst = QK8[:, :, 0, cs]
                if pick_engine(1024) == "A":
                    nc.scalar.activation(
                        dst, src_ap, mybir.ActivationFunctionType.Copy
                    )
                else:
                    nc.vector.tensor_copy(dst, src_ap)
                if sc % 2 == 1:
                    g = sc // 2
                    psv = pv.tile([128, 512], F32, tag="v")
                    for j in range(8):
                        mc = 8 * g + j
                        nc.tensor.matmul(
                            psv[:, 64 * j : 64 * j + HID + 1],
                            lhsT=XB[:, 128 * mc : 128 * mc + 128],
                            rhs=WVA,
                        )
                    vsrc = psv[:].rearrange("p (a b) -> p a b", b=64)[:, :, 0 : HID + 1]
                    vdst = VA[:, 4 * g : 4 * g + 4, :, :]
                    if pick_engine(136) == "A":
                        nc.scalar.activation(
                            vdst, vsrc, mybir.ActivationFunctionType.Copy
                        )
                    else:
                        nc.vector.tensor_copy(vdst, vsrc)
                    # f32 sums of this group's D-chunk V columns (for the
                    # delta add-back); D positions: g0 -> {5,7}, else odd
                    vt = psv[:].rearrange("p (a b) -> p b a", b=64)
                    dsl = (
                        vt[:, 0 : HID + 1, 5:8:2]
                        if g == 0
                        else vt[:, 0 : HID + 1, 1:8:2]
                    )
                    nc.vector.tensor_reduce(
                        VASD[:, g, :], dsl, axis=mybir.AxisListType.X,
                        op=mybir.AluOpType.add,
                    )
            nc.vector.tensor_reduce(
                VSD[:], VASD[:].rearrange("p g d -> p d g"),
                axis=mybir.AxisListType.X, op=mybir.AluOpType.add,
            )
            nc.vector.tensor_copy(VSDH[:], VSD[:])
            nc.vector.tensor_tensor(
                VSDL[:], VSD[:], VSDH[:], op=mybir.AluOpType.subtract
            )
            nc.gpsimd.memset(ONESB[:], 1.0)

        # ---- attention: S^T (fp8 DoubleRow) -> exp -> AV (transposed) ------
        with tc.tile_pool(name="s_pool", bufs=1, space="PSUM") as s_pool, \
             tc.tile_pool(name="av_pool", bufs=1, space="PSUM") as av_pool, \
             tc.tile_pool(name="pt_pool", bufs=pt_bufs) as pt_pool:
            # one [128, 17] accumulator per image row h, packed 16 per bank
            av = av_pool.tile([128, nh, 32], F32, tag="av", name="av")

            def av_mm(j, pt, h):
                nc.tensor.matmul(
                    av[:, h, 0 : HID + 1],
                    lhsT=pt[:, :, 128 * h : 128 * h + 128],
                    rhs=VA[:, j, :, :],
                    perf_mode=mybir.MatmulPerfMode.DoubleRow,
                    start=False,
                    stop=(j == npair - 1),
                    skip_group_check=True,
                )

            # seed each accumulator with sum_{m in D-chunks} V_aug[m, :]
            # (bf16 hi+lo ones-matmuls; delta-form chunks contribute P-1)
            for h in range(nh):
                # start=True only on the first write of each PSUM bank: a
                # start re-zeroes the whole bank's accumulation group, so
                # later regions must join with start=False
                nc.tensor.matmul(
                    av[:, h, 0 : HID + 1],
                    lhsT=ONESB[:],
                    rhs=VSDH[:],
                    start=(h % 16 == 0),
                    stop=False,
                    skip_group_check=True,
                )
                nc.tensor.matmul(
                    av[:, h, 0 : HID + 1],
                    lhsT=ONESB[:],
                    rhs=VSDL[:],
                    start=False,
                    stop=False,
                    skip_group_check=True,
                )

            def emit_slot(j, t, col, w, eng, pt):
                mc = 2 * j + t
                if eng == "A":
                    s = s_pool.tile([128, 1024], F32, tag="sa", bufs=2, name="sa")
                else:
                    s = s_pool.tile([128, 512], F32, tag="sd", bufs=2, name="sd")
                for sub in range(0, w, 512):
                    nc.tensor.matmul(
                        s[:, sub : sub + 512],
                        lhsT=QK8[:, 1, :, 128 * mc : 128 * mc + 128],
                        rhs=QK8[:, 0, :, col + sub : col + sub + 512],
                        perf_mode=mybir.MatmulPerfMode.DoubleRow,
                    )
                dstp = pt[:, t, col : col + w]
                if eng == "A":
                    nc.scalar.activation(
                        dstp,
                        s[:, 0:w],
                        mybir.ActivationFunctionType.Exp,
                        scale=1.0 / 1024.0,
                    )
                else:
                    nc.vector._custom_dve(
                        EXPM1_OP,
                        out=dstp,
                        in0=s[:, 0:w],
                        s0=EXPM1_C0,
                        s1=EXPM1_C1,
                        imm2=EXPM1_C2,
                    )

            # build per-engine slot streams (chunk -> engine fixed by
            # D_CHUNKS), then merge by projected engine finish time so the
            # PE feeds both rings concurrently
            streams = {"A": [], "D": []}
            slots_left = [0] * npair
            for j in range(npair):
                for t in range(2):
                    mc = 2 * j + t
                    eng = "D" if mc in D_CHUNKS else "A"
                    w = 1024 if eng == "A" else 512
                    for col in range(0, n_tok, w):
                        streams[eng].append((j, t, col, w))
                        slots_left[j] += 1

            pt_tiles = {}

            def get_pt(j):
                if j not in pt_tiles:
                    pt_tiles[j] = pt_pool.tile(
                        [128, 2, n_tok], FP8, tag="pt", name="pt"
                    )
                return pt_tiles[j]

            pend = []
            ii = {"A": 0, "D": 0}
            while ii["A"] < len(streams["A"]) or ii["D"] < len(streams["D"]):
                if ii["A"] >= len(streams["A"]):
                    eng = "D"
                elif ii["D"] >= len(streams["D"]):
                    eng = "A"
                else:
                    wa = streams["A"][ii["A"]][3]
                    wd = streams["D"][ii["D"]][3]
                    ca = eng_t["A"] + wa * ACT_RATE + ACT_INIT
                    cd = eng_t["D"] + wd * DVE_RATE + DVE_INIT
                    eng = "A" if ca <= cd else "D"
                j, t, col, w = streams[eng][ii[eng]]
                ii[eng] += 1
                charge_engine(eng, w)
                emit_slot(j, t, col, w, eng, get_pt(j))
                slots_left[j] -= 1
                if slots_left[j] == 0:
                    pend.extend((j, pt_tiles[j], h) for h in range(nh))
                while len(pend) > nh:
                    av_mm(*pend.pop(0))
            for args in pend:
                av_mm(*args)

            # ---- normalize: reciprocal of denominators + broadcast mult ----
            for b in range(nh // 16):
                hs = slice(16 * b, 16 * b + 16)
                nc.vector.reciprocal(RC[:, hs], av[:, hs, HID : HID + 1])
                nc.vector.tensor_tensor(
                    OF[:, hs, :],
                    av[:, hs, 0:HID],
                    RC[:, hs].unsqueeze(2).broadcast_to([128, 16, HID]),
                    op=mybir.AluOpType.mult,
                )

        # ---- final linear ---------------------------------------------------
        with tc.tile_pool(name="tail_ps", bufs=1, space="PSUM") as tail_ps:
            for b in range(nh // 16):
                fs = slice(256 * b, 256 * b + 256)
                psf = tail_ps.tile([OUT_DIM, 256], F32, tag="f", bufs=2, name="psf")
                nc.tensor.matmul(
                    psf[:], lhsT=WL[:], rhs=OF[:, 16 * b : 16 * b + 16, :]
                )
                nc.scalar.activation(
                    RES[:, fs], psf[:], mybir.ActivationFunctionType.Identity,
                    bias=BLB[:],
                )
                nc.sync.dma_start(out.ap()[:, fs], RES[:, fs])

    nc.compile()
    return nc


# ---------------------------------------------------------------------------
def make_core_inputs(x, wq, bq, wk, bk, wv, bv, w_lin, b_lin, n_tok=N_TOK):
    """Host-side prep: full inputs -> list of 8 per-core input dicts."""
    X = np.asarray(x, np.float32).reshape(C_IN, -1)[:, :n_tok]
    xa = np.ones((C_IN + 1, n_tok), np.float32)
    xa[:C_IN] = X
    xin = xa.astype(ml_dtypes.bfloat16)
    wlt = np.ascontiguousarray(np.asarray(w_lin, np.float32).T).astype(
        ml_dtypes.bfloat16
    )
    blb = np.asarray(b_lin, np.float32).reshape(OUT_DIM, 1)
    ztil = np.zeros((HID, n_tok), NPF8)

    s = QK_SCALE
    maps = []
    for h in range(HEADS):
        sl = slice(HID * h, HID * (h + 1))

        def aug(w, b, ones_col=False):
            d = HID + 1 if ones_col else HID
            m = np.zeros((C_IN + 1, d), np.float32)
            m[0:C_IN, 0:HID] = s * np.asarray(w, np.float32)[sl].T
            m[C_IN, 0:HID] = s * np.asarray(b, np.float32)[sl]
            if ones_col:
                m[C_IN, HID] = s
            return m.astype(ml_dtypes.bfloat16)

        wpk = np.concatenate(
            [
                np.asarray(aug(wq, bq), np.float32),
                np.asarray(aug(wk, bk), np.float32),
                np.asarray(aug(wv, bv, ones_col=True), np.float32),
            ],
            axis=1,
        ).astype(ml_dtypes.bfloat16)
        maps.append(
            {
                "xin": xin,
                "wpk": wpk,
                "ztil_v3": ztil,
                "wlt": wlt,
                "blb": blb,
            }
        )
    return maps


_MODULE_CACHE = {}


def _get_module(**kw):
    key = tuple(sorted(kw.items()))
    if key not in _MODULE_CACHE:
        _MODULE_CACHE[key] = build_module(**kw)
    return _MODULE_CACHE[key]


def kernel(x, wq, bq, wk, bk, wv, bv, w_lin, b_lin):
    from concourse.bass_utils import run_bass_kernel_spmd

    nc = _get_module()
    in_maps = make_core_inputs(x, wq, bq, wk, bk, wv, bv, w_lin, b_lin)
    res = run_bass_kernel_spmd(nc, in_maps, core_ids=list(range(N_CORES)))
    full = np.empty((1, HEADS * HID, H_IMG, OUT_DIM), np.float32)
    for h in range(HEADS):
        o = res.results[h]["out"].reshape(OUT_DIM, H_IMG, HID)
        full[0, HID * h : HID * (h + 1)] = o.transpose(2, 1, 0)
    return full


# revision 36
# speedup vs baseline: 1.6029x; 1.0121x over previous
"""Trainium2 Bass kernel for nn_MultiHeadSelfAttention2d (fp8 redesign).

Reference computation (B=1, C=64, H=32, W=128, HEADS=8, HIDDEN=16):
  q/k/v = 1x1 conv over channels (+bias), per-head attention over N=H*W=4096
  positions, softmax(q k^T / sqrt(16)), out = attn @ v, then a Linear over the
  W axis (W == HEADS*HIDDEN == 128) producing (1, 128, 32, 64).

Distribution: one head per NeuronCore -> 8 cores, fully independent.

Per-core dataflow:
  - proj:   V^T [m, d_aug] via X-stationary bf16 matmuls (bias + ones col
            folded into the augmented wva, all scaled x16), converted to
            fp8e4.  Q,K = W_aug x (bias row folded, x16) -> PSUM -> fp8e4
            in [d=16, (q|k), plane, n] layout where plane 1 is zeros.
  - S^T:    fp8 DoubleRow matmuls: lhsT = K[16,2,128] (plane1 = 0), rhs =
            Q[16,2,512] -> S^T*256 in PSUM at 0.5 cycles/col.
  - exp:    exp(u/1024) from PSUM -> fp8e4 P^T pair tiles [128, 2, n],
            split between ACT (hw exp) and DVE (cubic-poly custom op) by a
            greedy static schedule.
  - AV:     transposed accumulation: stationary = P^T pair [128,2,128],
            moving = V_aug pair [128,2,17] -> out [128(n=w), 17] per image
            row h, PSUM-accumulated over the 16 m-chunk pairs (DoubleRow).
            Output arrives already transposed; col 16 is the softmax
            denominator (V_aug ones column).
  - norm:   strided reciprocal of col 16, stride-0-broadcast tensor_tensor
            multiply -> OF [128(w), 32(h), 16(c)] bf16.
  - linear: out[o, (h,c)] = W_lin^T-stationary matmul over w + per-partition
            bias -> DMA out [64, 512] f32 (host transposes).
"""

import os
from contextlib import ExitStack

import ml_dtypes
import numpy as np

import concourse.bass as bass
import concourse.tile as tile
from concourse import bacc, mybir

# ---------------------------------------------------------------------------
# Problem constants (hardcoded per the task contract)
HEADS = 8
HID = 16
C_IN = 64
OUT_DIM = 64
H_IMG = 32
W_IMG = 128
N_TOK = H_IMG * W_IMG  # 4096
N_CORES = 8

BF16 = mybir.dt.bfloat16
F32 = mybir.dt.float32
FP8 = mybir.dt.float8e4
NPF8 = ml_dtypes.float8_e4m3

QK_SCALE = 16.0  # host-side scale on wq/wk/wv (and their biases)

# ---------------------------------------------------------------------------
# Custom DVE (vector engine) op: out = (((c3*u + c2)*u + c1)*u + 1)^2
# With c1=1/2048, c2=1/(2*2048^2), c3=1/(6*2048^3) this is exp(u/1024) to
# ~1e-5 rel for |u| < 220 (S*256 observed < 220).  Lets the Vector engine
# share softmax-exp work with the Scalar engine.
from concourse.dve_spec import Spec, Src0, C0, C1, C2, One, sq, lower
from concourse.dve_uop import DveOpSpec
from concourse import dve_ops
from concourse.dve_table_gen import dve_ver_for

EXP_C1 = 1.0 / 2048.0
EXP_C2 = 1.0 / (2.0 * 2048.0**2)
EXP_C3 = 1.0 / (6.0 * 2048.0**3)


def _exp_ref(in0, in1, c0, c1, c2):
    u = in0.astype(np.float32)
    q = ((np.float32(c2) * u + np.float32(c1)) * u + np.float32(c0)) * u + np.float32(
        1.0
    )
    return q * q


def _register_exp_op():
    name = "EXP_QTR_POLY_ANT"
    for op in dve_ops.OPS:
        if op.name == name:
            return op
    body = sq(((Src0 * C2 + C1) * Src0 + C0) * Src0 + One)
    spec = Spec(body=body, reference=_exp_ref)
    row = max(dve_ops._SUB_OPCODE_FOR_NAME.values()) + 1
    assert row < 0x20
    dve_ops._SUB_OPCODE_FOR_NAME[name] = row
    shas = {}
    for ver in ("v3", "v4"):
        try:
            uops = lower(spec, ver=ver)
            shas[ver] = DveOpSpec(name=name, opcode=row, uops=uops, rd1_en=False).sha(
                ver
            )
        except Exception:
            pass
    op = dve_ops.DveOp(name, spec, subdim=False, uops_sha=shas)
    dve_ops.OPS.append(op)
    dve_ops.CUSTOM_DVE_SPECS[name] = spec
    return op


EXP_OP = _register_exp_op()

# expm1 variant: out = ((c2*u + c1)*u + c0)*u  ~=  exp(u/1024) - 1
# (delta form keeps fp8 quantization error ~4x smaller near P=1)
EXPM1_C0 = 1.0 / 1024.0
EXPM1_C1 = 1.0 / 2097152.0
EXPM1_C2 = 4.0 / (3.0 * 2048.0**3)


def _expm1_ref(in0, in1, c0, c1, c2):
    u = in0.astype(np.float32)
    return ((np.float32(c2) * u + np.float32(c1)) * u + np.float32(c0)) * u


def _register_expm1_op():
    name = "EXPM1_CUBIC_ANT"
    for op in dve_ops.OPS:
        if op.name == name:
            return op
    body = ((Src0 * C2 + C1) * Src0 + C0) * Src0
    spec = Spec(body=body, reference=_expm1_ref)
    row = max(dve_ops._SUB_OPCODE_FOR_NAME.values()) + 1
    assert row < 0x20
    dve_ops._SUB_OPCODE_FOR_NAME[name] = row
    shas = {}
    for ver in ("v3", "v4"):
        try:
            uops = lower(spec, ver=ver)
            shas[ver] = DveOpSpec(name=name, opcode=row, uops=uops, rd1_en=False).sha(
                ver
            )
        except Exception:
            pass
    op = dve_ops.DveOp(name, spec, subdim=False, uops_sha=shas)
    dve_ops.OPS.append(op)
    dve_ops.CUSTOM_DVE_SPECS[name] = spec
    return op


EXPM1_OP = _register_expm1_op()

# m-chunks whose exp runs on DVE as delta=expm1 (odd chunks minus {1,3}):
# 14 of 32, balancing ACT 18*4*1038ns vs DVE 14*8*658ns.
D_CHUNKS = frozenset(mc for mc in range(1, 32, 2)) - {1, 3}


def _chunk_splits(n):
    """Split n columns into s-tile widths of 1024 (PSUM: 2 banks/tile)."""
    assert n % 1024 == 0
    return [(o, 1024) for o in range(0, n, 1024)]


# ---------------------------------------------------------------------------
def build_module(n_tok: int = N_TOK, act_bias_ns: float = -1000.0, pt_bufs: int = 5):
    """Builds (and bacc-compiles) the per-core Bass module."""
    assert n_tok % 1024 == 0
    mch = n_tok // 128  # m chunks (32)
    npair = mch // 2  # DoubleRow pairs (16)
    nh = n_tok // W_IMG  # image rows (32)
    f_tot = nh * HID  # (h,c) columns of the final output (512)

    nc = bacc.Bacc()

    xin = nc.dram_tensor("xin", [C_IN + 1, n_tok], BF16, kind="ExternalInput")
    wpk = nc.dram_tensor("wpk", [C_IN + 1, 2 * HID + HID + 1], BF16, kind="ExternalInput")
    ztil = nc.dram_tensor("ztil_v4", [HID, n_tok], FP8, kind="ExternalInput")
    wlt = nc.dram_tensor("wlt", [W_IMG, OUT_DIM], BF16, kind="ExternalInput")
    blb = nc.dram_tensor("blb", [OUT_DIM, 1], F32, kind="ExternalInput")
    out = nc.dram_tensor("out", [OUT_DIM, f_tot], F32, kind="ExternalOutput")

    # --- engine assignment: time-balanced greedy over per-engine rings -----
    ACT_RATE, ACT_INIT = 1.0 / 1.2, 185.0
    DVE_RATE, DVE_INIT = 1.0 / 0.96, 125.0
    eng_t = {"A": 1283.0 + act_bias_ns, "D": 0.0}

    def charge_engine(eng, width):
        if eng == "A":
            eng_t["A"] += width * ACT_RATE + ACT_INIT
        else:
            eng_t["D"] += width * DVE_RATE + DVE_INIT

    def pick_engine(width_a, width_d=None):
        """Pick engine for an op of width_a on ACT / width_d on DVE."""
        if width_d is None:
            width_d = width_a
        ca = width_a * ACT_RATE + ACT_INIT
        cd = width_d * DVE_RATE + DVE_INIT
        if eng_t["A"] + ca <= eng_t["D"] + cd:
            eng_t["A"] += ca
            return "A"
        eng_t["D"] += cd
        return "D"

    with tile.TileContext(nc) as tc, ExitStack() as ctx:
        const = ctx.enter_context(tc.tile_pool(name="const", bufs=1))

        WPK = const.tile([C_IN + 1, 2 * HID + HID + 1], BF16)
        nc.sync.dma_start(WPK[:], wpk.ap())
        WQA = WPK[:, 0:HID]
        WKA = WPK[:, HID : 2 * HID]
        WVA = WPK[:, 2 * HID : 3 * HID + 1]

        XB = const.tile([C_IN + 1, n_tok], BF16)
        QK8 = const.tile([HID, 2, 2, n_tok], FP8)
        WL = const.tile([W_IMG, OUT_DIM], BF16)
        BLB = const.tile([OUT_DIM, 1], F32)
        for q in range(4):
            qs = slice(q * (n_tok // 4), (q + 1) * (n_tok // 4))
            nc.sync.dma_start(XB[:, qs], xin.ap()[:, qs])
        nc.sync.dma_start(QK8[:, 0, 1, :], ztil.ap())
        nc.sync.dma_start(QK8[:, 1, 1, :], ztil.ap())
        nc.sync.dma_start(WL[:], wlt.ap())
        nc.sync.dma_start(BLB[:], blb.ap())

        VA = const.tile([128, npair, 2, HID + 1], FP8)
        VASD = const.tile([128, 4, HID + 1], F32)  # per-group D-chunk V sums
        VSD = const.tile([128, HID + 1], F32)  # sum over all D-chunk m of V_aug
        VSDH = const.tile([128, HID + 1], BF16)
        VSDL = const.tile([128, HID + 1], BF16)
        ONESB = const.tile([128, 128], BF16)
        RC = const.tile([128, nh], F32)
        OF = const.tile([128, nh, HID], BF16)
        RES = const.tile([OUT_DIM, f_tot], F32)

        # ---- interleaved Q/K + V projection, fp8 convert ------------------
        with tc.tile_pool(name="pv", bufs=2, space="PSUM") as pv, \
             tc.tile_pool(name="pqk", bufs=2, space="PSUM") as pqk:
            for sc in range(n_tok // 512):
                ps = pqk.tile([HID, 1024], F32, tag="qk")
                cs = slice(512 * sc, 512 * sc + 512)
                nc.tensor.matmul(ps[:, 0:512], lhsT=WQA, rhs=XB[:, cs])
                nc.tensor.matmul(ps[:, 512:1024], lhsT=WKA, rhs=XB[:, cs])
                src_ap = ps[:].rearrange("p (a b) -> p a b", b=512)
                dst = QK8[:, :, 0, cs]
                if pick_engine(1024) == "A":
                    nc.scalar.activation(
                        dst, src_ap, mybir.ActivationFunctionType.Copy
                    )
                else:
                    nc.vector.tensor_copy(dst, src_ap)
                if sc % 2 == 1:
                    g = sc // 2
                    psv = pv.tile([128, 512], F32, tag="v")
                    for j in range(8):
                        mc = 8 * g + j
                        nc.tensor.matmul(
                            psv[:, 64 * j : 64 * j + HID + 1],
                            lhsT=XB[:, 128 * mc : 128 * mc + 128],
                            rhs=WVA,
                        )
                    vsrc = psv[:].rearrange("p (a b) -> p a b", b=64)[:, :, 0 : HID + 1]
                    vdst = VA[:, 4 * g : 4 * g + 4, :, :]
                    if pick_engine(136) == "A":
                        nc.scalar.activation(
                            vdst, vsrc, mybir.ActivationFunctionType.Copy
                        )
                    else:
                        nc.vector.tensor_copy(vdst, vsrc)
                    # f32 sums of this group's D-chunk V columns (for the
                    # delta add-back); D positions: g0 -> {5,7}, else odd
                    vt = psv[:].rearrange("p (a b) -> p b a", b=64)
                    dsl = (
                        vt[:, 0 : HID + 1, 5:8:2]
                        if g == 0
                        else vt[:, 0 : HID + 1, 1:8:2]
                    )
                    nc.vector.tensor_reduce(
                        VASD[:, g, :], dsl, axis=mybir.AxisListType.X,
                        op=mybir.AluOpType.add,
                    )
            nc.vector.tensor_reduce(
                VSD[:], VASD[:].rearrange("p g d -> p d g"),
                axis=mybir.AxisListType.X, op=mybir.AluOpType.add,
            )
            nc.vector.tensor_copy(VSDH[:], VSD[:])
            nc.vector.tensor_tensor(
                VSDL[:], VSD[:], VSDH[:], op=mybir.AluOpType.subtract
            )
            nc.gpsimd.memset(ONESB[:], 1.0)

        # ---- attention: S^T (fp8 DoubleRow) -> exp -> AV (transposed) ------
        with tc.tile_pool(name="s_pool", bufs=1, space="PSUM") as s_pool, \
             tc.tile_pool(name="av_pool", bufs=1, space="PSUM") as av_pool, \
             tc.tile_pool(name="pt_pool", bufs=pt_bufs) as pt_pool:
            # one [128, 17] accumulator per image row h, packed 16 per bank
            av = av_pool.tile([128, nh, 32], F32, tag="av", name="av")

            def av_mm(j, pt, h):
                nc.tensor.matmul(
                    av[:, h, 0 : HID + 1],
                    lhsT=pt[:, :, 128 * h : 128 * h + 128],
                    rhs=VA[:, j, :, :],
                    perf_mode=mybir.MatmulPerfMode.DoubleRow,
                    start=False,
                    stop=(j == npair - 1),
                    skip_group_check=True,
                )

            # seed each accumulator with sum_{m in D-chunks} V_aug[m, :]
            # (bf16 hi+lo ones-matmuls; delta-form chunks contribute P-1)
            for h in range(nh):
                # start=True only on the first write of each PSUM bank: a
                # start re-zeroes the whole bank's accumulation group, so
                # later regions must join with start=False
                nc.tensor.matmul(
                    av[:, h, 0 : HID + 1],
                    lhsT=ONESB[:],
                    rhs=VSDH[:],
                    start=(h % 16 == 0),
                    stop=False,
                    skip_group_check=True,
                )
                nc.tensor.matmul(
                    av[:, h, 0 : HID + 1],
                    lhsT=ONESB[:],
                    rhs=VSDL[:],
                    start=False,
                    stop=False,
                    skip_group_check=True,
                )

            def emit_slot(j, t, col, w, eng, pt):
                mc = 2 * j + t
                if eng == "A":
                    s = s_pool.tile([128, 1024], F32, tag="sa", bufs=2, name="sa")
                else:
                    s = s_pool.tile([128, 512], F32, tag="sd", bufs=2, name="sd")
                for sub in range(0, w, 512):
                    nc.tensor.matmul(
                        s[:, sub : sub + 512],
                        lhsT=QK8[:, 1, :, 128 * mc : 128 * mc + 128],
                        rhs=QK8[:, 0, :, col + sub : col + sub + 512],
                        perf_mode=mybir.MatmulPerfMode.DoubleRow,
                    )
                dstp = pt[:, t, col : col + w]
                if eng == "A":
                    nc.scalar.activation(
                        dstp,
                        s[:, 0:w],
                        mybir.ActivationFunctionType.Exp,
                        scale=1.0 / 1024.0,
                    )
                else:
                    nc.vector._custom_dve(
                        EXPM1_OP,
                        out=dstp,
                        in0=s[:, 0:w],
                        s0=EXPM1_C0,
                        s1=EXPM1_C1,
                        imm2=EXPM1_C2,
                    )

            # build per-engine slot streams (chunk -> engine fixed by
            # D_CHUNKS), then merge by projected engine finish time so the
            # PE feeds both rings concurrently
            streams = {"A": [], "D": []}
            slots_left = [0] * npair
            for j in range(npair):
                for t in range(2):
                    mc = 2 * j + t
                    eng = "D" if mc in D_CHUNKS else "A"
                    w = 1024 if eng == "A" else 512
                    for col in range(0, n_tok, w):
                        streams[eng].append((j, t, col, w))
                        slots_left[j] += 1

            pt_tiles = {}

            def get_pt(j):
                if j not in pt_tiles:
                    pt_tiles[j] = pt_pool.tile(
                        [128, 2, n_tok], FP8, tag="pt", name="pt"
                    )
                return pt_tiles[j]

            pend = []
            ii = {"A": 0, "D": 0}
            while ii["A"] < len(streams["A"]) or ii["D"] < len(streams["D"]):
                if ii["A"] >= len(streams["A"]):
                    eng = "D"
                elif ii["D"] >= len(streams["D"]):
                    eng = "A"
                else:
                    wa = streams["A"][ii["A"]][3]
                    wd = streams["D"][ii["D"]][3]
                    ca = eng_t["A"] + wa * ACT_RATE + ACT_INIT
                    cd = eng_t["D"] + wd * DVE_RATE + DVE_INIT
                    eng = "A" if ca <= cd else "D"
                j, t, col, w = streams[eng][ii[eng]]
                ii[eng] += 1
                charge_engine(eng, w)
                emit_slot(j, t, col, w, eng, get_pt(j))
                slots_left[j] -= 1
                if slots_left[j] == 0:
                    pend.extend((j, pt_tiles[j], h) for h in range(nh))
                while len(pend) > nh:
                    av_mm(*pend.pop(0))
            for args in pend:
                av_mm(*args)

            # ---- normalize: reciprocal of denominators + broadcast mult ----
            for b in range(nh // 16):
                hs = slice(16 * b, 16 * b + 16)
                nc.vector.reciprocal(RC[:, hs], av[:, hs, HID : HID + 1])
                nc.vector.tensor_tensor(
                    OF[:, hs, :],
                    av[:, hs, 0:HID],
                    RC[:, hs].unsqueeze(2).broadcast_to([128, 16, HID]),
                    op=mybir.AluOpType.mult,
                )

        # ---- final linear ---------------------------------------------------
        with tc.tile_pool(name="tail_ps", bufs=1, space="PSUM") as tail_ps:
            for b in range(nh // 16):
                fs = slice(256 * b, 256 * b + 256)
                psf = tail_ps.tile([OUT_DIM, 256], F32, tag="f", bufs=2, name="psf")
                nc.tensor.matmul(
                    psf[:], lhsT=WL[:], rhs=OF[:, 16 * b : 16 * b + 16, :]
                )
                nc.scalar.activation(
                    RES[:, fs], psf[:], mybir.ActivationFunctionType.Identity,
                    bias=BLB[:],
                )
                nc.sync.dma_start(out.ap()[:, fs], RES[:, fs])

    nc.compile()
    return nc


# ---------------------------------------------------------------------------
def make_core_inputs(x, wq, bq, wk, bk, wv, bv, w_lin, b_lin, n_tok=N_TOK):
    """Host-side prep: full inputs -> list of 8 per-core input dicts."""
    X = np.asarray(x, np.float32).reshape(C_IN, -1)[:, :n_tok]
    xa = np.ones((C_IN + 1, n_tok), np.float32)
    xa[:C_IN] = X
    xin = xa.astype(ml_dtypes.bfloat16)
    wlt = np.ascontiguousarray(np.asarray(w_lin, np.float32).T).astype(
        ml_dtypes.bfloat16
    )
    blb = np.asarray(b_lin, np.float32).reshape(OUT_DIM, 1)
    ztil = np.zeros((HID, n_tok), NPF8)

    s = QK_SCALE
    maps = []
    for h in range(HEADS):
        sl = slice(HID * h, HID * (h + 1))

        def aug(w, b, ones_col=False):
            d = HID + 1 if ones_col else HID
            m = np.zeros((C_IN + 1, d), np.float32)
            m[0:C_IN, 0:HID] = s * np.asarray(w, np.float32)[sl].T
            m[C_IN, 0:HID] = s * np.asarray(b, np.float32)[sl]
            if ones_col:
                m[C_IN, HID] = s
            return m.astype(ml_dtypes.bfloat16)

        wpk = np.concatenate(
            [
                np.asarray(aug(wq, bq), np.float32),
                np.asarray(aug(wk, bk), np.float32),
                np.asarray(aug(wv, bv, ones_col=True), np.float32),
            ],
            axis=1,
        ).astype(ml_dtypes.bfloat16)
        maps.append(
            {
                "xin": xin,
                "wpk": wpk,
                "ztil_v4": ztil,
                "wlt": wlt,
                "blb": blb,
            }
        )
    return maps


_MODULE_CACHE = {}


def _get_module(**kw):
    key = tuple(sorted(kw.items()))
    if key not in _MODULE_CACHE:
        _MODULE_CACHE[key] = build_module(**kw)
    return _MODULE_CACHE[key]


def kernel(x, wq, bq, wk, bk, wv, bv, w_lin, b_lin):
    from concourse.bass_utils import run_bass_kernel_spmd

    nc = _get_module()
    in_maps = make_core_inputs(x, wq, bq, wk, bk, wv, bv, w_lin, b_lin)
    res = run_bass_kernel_spmd(nc, in_maps, core_ids=list(range(N_CORES)))
    full = np.empty((1, HEADS * HID, H_IMG, OUT_DIM), np.float32)
    for h in range(HEADS):
        o = res.results[h]["out"].reshape(OUT_DIM, H_IMG, HID)
        full[0, HID * h : HID * (h + 1)] = o.transpose(2, 1, 0)
    return full


# revision 44
# speedup vs baseline: 1.6069x; 1.0025x over previous
"""Trainium2 Bass kernel for nn_MultiHeadSelfAttention2d (fp8 redesign).

Reference computation (B=1, C=64, H=32, W=128, HEADS=8, HIDDEN=16):
  q/k/v = 1x1 conv over channels (+bias), per-head attention over N=H*W=4096
  positions, softmax(q k^T / sqrt(16)), out = attn @ v, then a Linear over the
  W axis (W == HEADS*HIDDEN == 128) producing (1, 128, 32, 64).

Distribution: one head per NeuronCore -> 8 cores, fully independent.

Per-core dataflow:
  - proj:   V^T [m, d_aug] via X-stationary bf16 matmuls (bias + ones col
            folded into the augmented wva, all scaled x16), converted to
            fp8e4.  Q,K = W_aug x (bias row folded, x16) -> PSUM -> fp8e4
            in [d=16, (q|k), plane, n] layout where plane 1 is zeros.
  - S^T:    fp8 DoubleRow matmuls: lhsT = K[16,2,128] (plane1 = 0), rhs =
            Q[16,2,512] -> S^T*256 in PSUM at 0.5 cycles/col.
  - exp:    exp(u/1024) from PSUM -> fp8e4 P^T pair tiles [128, 2, n],
            split between ACT (hw exp) and DVE (cubic-poly custom op) by a
            greedy static schedule.
  - AV:     transposed accumulation: stationary = P^T pair [128,2,128],
            moving = V_aug pair [128,2,17] -> out [128(n=w), 17] per image
            row h, PSUM-accumulated over the 16 m-chunk pairs (DoubleRow).
            Output arrives already transposed; col 16 is the softmax
            denominator (V_aug ones column).
  - norm:   strided reciprocal of col 16, stride-0-broadcast tensor_tensor
            multiply -> OF [128(w), 32(h), 16(c)] bf16.
  - linear: out[o, (h,c)] = W_lin^T-stationary matmul over w + per-partition
            bias -> DMA out [64, 512] f32 (host transposes).
"""

import os
from contextlib import ExitStack

import ml_dtypes
import numpy as np

import concourse.bass as bass
import concourse.tile as tile
from concourse import bacc, mybir

# ---------------------------------------------------------------------------
# Problem constants (hardcoded per the task contract)
HEADS = 8
HID = 16
C_IN = 64
OUT_DIM = 64
H_IMG = 32
W_IMG = 128
N_TOK = H_IMG * W_IMG  # 4096
N_CORES = 8

BF16 = mybir.dt.bfloat16
F32 = mybir.dt.float32
FP8 = mybir.dt.float8e4
NPF8 = ml_dtypes.float8_e4m3

QK_SCALE = 16.0  # host-side scale on wq/wk/wv (and their biases)

# ---------------------------------------------------------------------------
# Custom DVE (vector engine) op: out = (((c3*u + c2)*u + c1)*u + 1)^2
# With c1=1/2048, c2=1/(2*2048^2), c3=1/(6*2048^3) this is exp(u/1024) to
# ~1e-5 rel for |u| < 220 (S*256 observed < 220).  Lets the Vector engine
# share softmax-exp work with the Scalar engine.
from concourse.dve_spec import Spec, Src0, C0, C1, C2, One, sq, lower
from concourse.dve_uop import DveOpSpec
from concourse import dve_ops
from concourse.dve_table_gen import dve_ver_for

EXP_C1 = 1.0 / 2048.0
EXP_C2 = 1.0 / (2.0 * 2048.0**2)
EXP_C3 = 1.0 / (6.0 * 2048.0**3)


def _exp_ref(in0, in1, c0, c1, c2):
    u = in0.astype(np.float32)
    q = ((np.float32(c2) * u + np.float32(c1)) * u + np.float32(c0)) * u + np.float32(
        1.0
    )
    return q * q


def _register_exp_op():
    name = "EXP_QTR_POLY_ANT"
    for op in dve_ops.OPS:
        if op.name == name:
            return op
    body = sq(((Src0 * C2 + C1) * Src0 + C0) * Src0 + One)
    spec = Spec(body=body, reference=_exp_ref)
    row = max(dve_ops._SUB_OPCODE_FOR_NAME.values()) + 1
    assert row < 0x20
    dve_ops._SUB_OPCODE_FOR_NAME[name] = row
    shas = {}
    for ver in ("v3", "v4"):
        try:
            uops = lower(spec, ver=ver)
            shas[ver] = DveOpSpec(name=name, opcode=row, uops=uops, rd1_en=False).sha(
                ver
            )
        except Exception:
            pass
    op = dve_ops.DveOp(name, spec, subdim=False, uops_sha=shas)
    dve_ops.OPS.append(op)
    dve_ops.CUSTOM_DVE_SPECS[name] = spec
    return op


EXP_OP = _register_exp_op()

# expm1 variant: out = ((c2*u + c1)*u + c0)*u  ~=  exp(u/1024) - 1
# (delta form keeps fp8 quantization error ~4x smaller near P=1)
EXPM1_C0 = 1.0 / 1024.0
EXPM1_C1 = 1.0 / 2097152.0
EXPM1_C2 = 4.0 / (3.0 * 2048.0**3)


def _expm1_ref(in0, in1, c0, c1, c2):
    u = in0.astype(np.float32)
    return ((np.float32(c2) * u + np.float32(c1)) * u + np.float32(c0)) * u


def _register_expm1_op():
    name = "EXPM1_CUBIC_ANT"
    for op in dve_ops.OPS:
        if op.name == name:
            return op
    body = ((Src0 * C2 + C1) * Src0 + C0) * Src0
    spec = Spec(body=body, reference=_expm1_ref)
    row = max(dve_ops._SUB_OPCODE_FOR_NAME.values()) + 1
    assert row < 0x20
    dve_ops._SUB_OPCODE_FOR_NAME[name] = row
    shas = {}
    for ver in ("v3", "v4"):
        try:
            uops = lower(spec, ver=ver)
            shas[ver] = DveOpSpec(name=name, opcode=row, uops=uops, rd1_en=False).sha(
                ver
            )
        except Exception:
            pass
    op = dve_ops.DveOp(name, spec, subdim=False, uops_sha=shas)
    dve_ops.OPS.append(op)
    dve_ops.CUSTOM_DVE_SPECS[name] = spec
    return op


EXPM1_OP = _register_expm1_op()

# m-chunks whose exp runs on DVE as delta=expm1 (odd chunks minus {1,3}):
# 14 of 32, balancing ACT 18*4*1038ns vs DVE 14*8*658ns.
D_CHUNKS = frozenset(mc for mc in range(1, 32, 2)) - {1, 3}


def _chunk_splits(n):
    """Split n columns into s-tile widths of 1024 (PSUM: 2 banks/tile)."""
    assert n % 1024 == 0
    return [(o, 1024) for o in range(0, n, 1024)]


# ---------------------------------------------------------------------------
def build_module(n_tok: int = N_TOK, act_bias_ns: float = -1000.0, pt_bufs: int = 5, pend_max: int = 32):
    """Builds (and bacc-compiles) the per-core Bass module."""
    assert n_tok % 1024 == 0
    mch = n_tok // 128  # m chunks (32)
    npair = mch // 2  # DoubleRow pairs (16)
    nh = n_tok // W_IMG  # image rows (32)
    f_tot = nh * HID  # (h,c) columns of the final output (512)

    nc = bacc.Bacc()

    xin = nc.dram_tensor("xin", [C_IN + 1, n_tok], BF16, kind="ExternalInput")
    wpk = nc.dram_tensor("wpk", [C_IN + 1, 2 * HID + HID + 1], BF16, kind="ExternalInput")
    ztil = nc.dram_tensor("ztil_v5", [HID, n_tok], FP8, kind="ExternalInput")
    wlt = nc.dram_tensor("wlt", [W_IMG, OUT_DIM], BF16, kind="ExternalInput")
    blb = nc.dram_tensor("blb", [OUT_DIM, 1], F32, kind="ExternalInput")
    out = nc.dram_tensor("out", [OUT_DIM, f_tot], F32, kind="ExternalOutput")

    # --- engine assignment: time-balanced greedy over per-engine rings -----
    ACT_RATE, ACT_INIT = 1.0 / 1.2, 185.0
    DVE_RATE, DVE_INIT = 1.0 / 0.96, 125.0
    eng_t = {"A": 1283.0 + act_bias_ns, "D": 0.0}

    def charge_engine(eng, width):
        if eng == "A":
            eng_t["A"] += width * ACT_RATE + ACT_INIT
        else:
            eng_t["D"] += width * DVE_RATE + DVE_INIT

    def pick_engine(width_a, width_d=None):
        """Pick engine for an op of width_a on ACT / width_d on DVE."""
        if width_d is None:
            width_d = width_a
        ca = width_a * ACT_RATE + ACT_INIT
        cd = width_d * DVE_RATE + DVE_INIT
        if eng_t["A"] + ca <= eng_t["D"] + cd:
            eng_t["A"] += ca
            return "A"
        eng_t["D"] += cd
        return "D"

    with tile.TileContext(nc) as tc, ExitStack() as ctx:
        const = ctx.enter_context(tc.tile_pool(name="const", bufs=1))

        WPK = const.tile([C_IN + 1, 2 * HID + HID + 1], BF16)
        nc.sync.dma_start(WPK[:], wpk.ap())
        WQA = WPK[:, 0:HID]
        WKA = WPK[:, HID : 2 * HID]
        WVA = WPK[:, 2 * HID : 3 * HID + 1]

        XB = const.tile([C_IN + 1, n_tok], BF16)
        QK8 = const.tile([HID, 2, 2, n_tok], FP8)
        WL = const.tile([W_IMG, OUT_DIM], BF16)
        BLB = const.tile([OUT_DIM, 1], F32)
        for q0, q1 in [(0, 512), (512, 1024), (1024, 2048), (2048, 3072), (3072, 4096)]:
            q0 = q0 * n_tok // 4096
            q1 = q1 * n_tok // 4096
            nc.sync.dma_start(XB[:, q0:q1], xin.ap()[:, q0:q1])
        nc.sync.dma_start(QK8[:, 0, 1, :], ztil.ap())
        nc.sync.dma_start(QK8[:, 1, 1, :], ztil.ap())
        nc.sync.dma_start(WL[:], wlt.ap())
        nc.sync.dma_start(BLB[:], blb.ap())

        VA = const.tile([128, npair, 2, HID + 1], FP8)
        VASD = const.tile([128, 4, HID + 1], F32)  # per-group D-chunk V sums
        VSD = const.tile([128, HID + 1], F32)  # sum over all D-chunk m of V_aug
        VSDH = const.tile([128, HID + 1], BF16)
        VSDL = const.tile([128, HID + 1], BF16)
        ONESB = const.tile([128, 128], BF16)
        RC = const.tile([128, nh], F32)
        OF = const.tile([128, nh, HID], BF16)
        RES = const.tile([OUT_DIM, f_tot], F32)

        # ---- interleaved Q/K + V projection, fp8 convert ------------------
        with tc.tile_pool(name="pv", bufs=2, space="PSUM") as pv, \
             tc.tile_pool(name="pqk", bufs=2, space="PSUM") as pqk:
            for sc in range(n_tok // 512):
                ps = pqk.tile([HID, 1024], F32, tag="qk")
                cs = slice(512 * sc, 512 * sc + 512)
                nc.tensor.matmul(ps[:, 0:512], lhsT=WQA, rhs=XB[:, cs])
                nc.tensor.matmul(ps[:, 512:1024], lhsT=WKA, rhs=XB[:, cs])
                src_ap = ps[:].rearrange("p (a b) -> p a b", b=512)
                dst = QK8[:, :, 0, cs]
                if pick_engine(1024) == "A":
                    nc.scalar.activation(
                        dst, src_ap, mybir.ActivationFunctionType.Copy
                    )
                else:
                    nc.vector.tensor_copy(dst, src_ap)
                if sc % 2 == 1:
                    g = sc // 2
                    psv = pv.tile([128, 512], F32, tag="v")
                    for j in range(8):
                        mc = 8 * g + j
                        nc.tensor.matmul(
                            psv[:, 64 * j : 64 * j + HID + 1],
                            lhsT=XB[:, 128 * mc : 128 * mc + 128],
                            rhs=WVA,
                        )
                    vsrc = psv[:].rearrange("p (a b) -> p a b", b=64)[:, :, 0 : HID + 1]
                    vdst = VA[:, 4 * g : 4 * g + 4, :, :]
                    if pick_engine(136) == "A":
                        nc.scalar.activation(
                            vdst, vsrc, mybir.ActivationFunctionType.Copy
                        )
                    else:
                        nc.vector.tensor_copy(vdst, vsrc)
                    # f32 sums of this group's D-chunk V columns (for the
                    # delta add-back); D positions: g0 -> {5,7}, else odd
                    vt = psv[:].rearrange("p (a b) -> p b a", b=64)
                    dsl = (
                        vt[:, 0 : HID + 1, 5:8:2]
                        if g == 0
                        else vt[:, 0 : HID + 1, 1:8:2]
                    )
                    nc.vector.tensor_reduce(
                        VASD[:, g, :], dsl, axis=mybir.AxisListType.X,
                        op=mybir.AluOpType.add,
                    )
            nc.vector.tensor_reduce(
                VSD[:], VASD[:].rearrange("p g d -> p d g"),
                axis=mybir.AxisListType.X, op=mybir.AluOpType.add,
            )
            nc.vector.tensor_copy(VSDH[:], VSD[:])
            nc.vector.tensor_tensor(
                VSDL[:], VSD[:], VSDH[:], op=mybir.AluOpType.subtract
            )
            nc.gpsimd.memset(ONESB[:], 1.0)

        # ---- attention: S^T (fp8 DoubleRow) -> exp -> AV (transposed) ------
        with tc.tile_pool(name="s_pool", bufs=1, space="PSUM") as s_pool, \
             tc.tile_pool(name="av_pool", bufs=1, space="PSUM") as av_pool, \
             tc.tile_pool(name="pt_pool", bufs=pt_bufs) as pt_pool:
            # one [128, 17] accumulator per image row h, packed 16 per bank
            av = av_pool.tile([128, nh, 32], F32, tag="av", name="av")

            def av_mm(j, pt, h):
                nc.tensor.matmul(
                    av[:, h, 0 : HID + 1],
                    lhsT=pt[:, :, 128 * h : 128 * h + 128],
                    rhs=VA[:, j, :, :],
                    perf_mode=mybir.MatmulPerfMode.DoubleRow,
                    start=False,
                    stop=(j == npair - 1),
                    skip_group_check=True,
                )

            # seed each accumulator with sum_{m in D-chunks} V_aug[m, :]
            # (bf16 hi+lo ones-matmuls; delta-form chunks contribute P-1)
            for h in range(nh):
                # start=True only on the first write of each PSUM bank: a
                # start re-zeroes the whole bank's accumulation group, so
                # later regions must join with start=False
                nc.tensor.matmul(
                    av[:, h, 0 : HID + 1],
                    lhsT=ONESB[:],
                    rhs=VSDH[:],
                    start=(h % 16 == 0),
                    stop=False,
                    skip_group_check=True,
                )
                nc.tensor.matmul(
                    av[:, h, 0 : HID + 1],
                    lhsT=ONESB[:],
                    rhs=VSDL[:],
                    start=False,
                    stop=False,
                    skip_group_check=True,
                )

            def emit_slot(j, t, col, w, eng, pt):
                mc = 2 * j + t
                if eng == "A":
                    s = s_pool.tile([128, 1024], F32, tag="sa", bufs=2, name="sa")
                else:
                    s = s_pool.tile([128, 512], F32, tag="sd", bufs=2, name="sd")
                for sub in range(0, w, 512):
                    nc.tensor.matmul(
                        s[:, sub : sub + 512],
                        lhsT=QK8[:, 1, :, 128 * mc : 128 * mc + 128],
                        rhs=QK8[:, 0, :, col + sub : col + sub + 512],
                        perf_mode=mybir.MatmulPerfMode.DoubleRow,
                    )
                dstp = pt[:, t, col : col + w]
                if eng == "A":
                    nc.scalar.activation(
                        dstp,
                        s[:, 0:w],
                        mybir.ActivationFunctionType.Exp,
                        scale=1.0 / 1024.0,
                    )
                else:
                    nc.vector._custom_dve(
                        EXPM1_OP,
                        out=dstp,
                        in0=s[:, 0:w],
                        s0=EXPM1_C0,
                        s1=EXPM1_C1,
                        imm2=EXPM1_C2,
                    )

            # build per-engine slot streams (chunk -> engine fixed by
            # D_CHUNKS), then merge by projected engine finish time so the
            # PE feeds both rings concurrently
            streams = {"A": [], "D": []}
            slots_left = [0] * npair
            for j in range(npair):
                for t in range(2):
                    mc = 2 * j + t
                    eng = "D" if mc in D_CHUNKS else "A"
                    w = 1024 if eng == "A" else 512
                    for col in range(0, n_tok, w):
                        streams[eng].append((j, t, col, w))
                        slots_left[j] += 1

            pt_tiles = {}

            def get_pt(j):
                if j not in pt_tiles:
                    pt_tiles[j] = pt_pool.tile(
                        [128, 2, n_tok], FP8, tag="pt", name="pt"
                    )
                return pt_tiles[j]

            pend = []
            ii = {"A": 0, "D": 0}
            lastj = npair - 1
            prog = [0, 0]  # column progress of the last pair's two planes
            next_h = 0  # next last-pair AV row to emit
            while ii["A"] < len(streams["A"]) or ii["D"] < len(streams["D"]):
                if ii["A"] >= len(streams["A"]):
                    eng = "D"
                elif ii["D"] >= len(streams["D"]):
                    eng = "A"
                else:
                    wa = streams["A"][ii["A"]][3]
                    wd = streams["D"][ii["D"]][3]
                    ca = eng_t["A"] + wa * ACT_RATE + ACT_INIT
                    cd = eng_t["D"] + wd * DVE_RATE + DVE_INIT
                    eng = "A" if ca <= cd else "D"
                j, t, col, w = streams[eng][ii[eng]]
                ii[eng] += 1
                charge_engine(eng, w)
                emit_slot(j, t, col, w, eng, get_pt(j))
                slots_left[j] -= 1
                if slots_left[j] == 0 and j != lastj:
                    pend.extend((j, pt_tiles[j], h) for h in range(nh))
                if j == lastj:
                    # progressive tail: drain earlier pairs, then emit each
                    # last-pair AV stop as soon as both planes cover its row
                    for args in pend:
                        av_mm(*args)
                    pend = []
                    prog[t] = col + w
                    lim = min(prog) // W_IMG
                    while next_h < lim:
                        av_mm(lastj, get_pt(lastj), next_h)
                        next_h += 1
                else:
                    while len(pend) > pend_max:
                        av_mm(*pend.pop(0))
            while next_h < nh:
                av_mm(lastj, get_pt(lastj), next_h)
                next_h += 1
            for args in pend:
                av_mm(*args)

            # ---- normalize: reciprocal of denominators + broadcast mult ----
            for b in range(nh // 16):
                hs = slice(16 * b, 16 * b + 16)
                nc.vector.reciprocal(RC[:, hs], av[:, hs, HID : HID + 1])
                nc.vector.tensor_tensor(
                    OF[:, hs, :],
                    av[:, hs, 0:HID],
                    RC[:, hs].unsqueeze(2).broadcast_to([128, 16, HID]),
                    op=mybir.AluOpType.mult,
                )

        # ---- final linear ---------------------------------------------------
        with tc.tile_pool(name="tail_ps", bufs=1, space="PSUM") as tail_ps:
            for b in range(nh // 16):
                fs = slice(256 * b, 256 * b + 256)
                psf = tail_ps.tile([OUT_DIM, 256], F32, tag="f", bufs=2, name="psf")
                nc.tensor.matmul(
                    psf[:], lhsT=WL[:], rhs=OF[:, 16 * b : 16 * b + 16, :]
                )
                nc.scalar.activation(
                    RES[:, fs], psf[:], mybir.ActivationFunctionType.Identity,
                    bias=BLB[:],
                )
                nc.sync.dma_start(out.ap()[:, fs], RES[:, fs])

    nc.compile()
    return nc


# ---------------------------------------------------------------------------
def make_core_inputs(x, wq, bq, wk, bk, wv, bv, w_lin, b_lin, n_tok=N_TOK):
    """Host-side prep: full inputs -> list of 8 per-core input dicts."""
    X = np.asarray(x, np.float32).reshape(C_IN, -1)[:, :n_tok]
    xa = np.ones((C_IN + 1, n_tok), np.float32)
    xa[:C_IN] = X
    xin = xa.astype(ml_dtypes.bfloat16)
    wlt = np.ascontiguousarray(np.asarray(w_lin, np.float32).T).astype(
        ml_dtypes.bfloat16
    )
    blb = np.asarray(b_lin, np.float32).reshape(OUT_DIM, 1)
    ztil = np.zeros((HID, n_tok), NPF8)

    s = QK_SCALE
    maps = []
    for h in range(HEADS):
        sl = slice(HID * h, HID * (h + 1))

        def aug(w, b, ones_col=False):
            d = HID + 1 if ones_col else HID
            m = np.zeros((C_IN + 1, d), np.float32)
            m[0:C_IN, 0:HID] = s * np.asarray(w, np.float32)[sl].T
            m[C_IN, 0:HID] = s * np.asarray(b, np.float32)[sl]
            if ones_col:
                m[C_IN, HID] = s
            return m.astype(ml_dtypes.bfloat16)

        wpk = np.concatenate(
            [
                np.asarray(aug(wq, bq), np.float32),
                np.asarray(aug(wk, bk), np.float32),
                np.asarray(aug(wv, bv, ones_col=True), np.float32),
            ],
            axis=1,
        ).astype(ml_dtypes.bfloat16)
        maps.append(
            {
                "xin": xin,
                "wpk": wpk,
                "ztil_v5": ztil,
                "wlt": wlt,
                "blb": blb,
            }
        )
    return maps


_MODULE_CACHE = {}


def _get_module(**kw):
    key = tuple(sorted(kw.items()))
    if key not in _MODULE_CACHE:
        _MODULE_CACHE[key] = build_module(**kw)
    return _MODULE_CACHE[key]


def kernel(x, wq, bq, wk, bk, wv, bv, w_lin, b_lin):
    from concourse.bass_utils import run_bass_kernel_spmd

    nc = _get_module()
    in_maps = make_core_inputs(x, wq, bq, wk, bk, wv, bv, w_lin, b_lin)
    res = run_bass_kernel_spmd(nc, in_maps, core_ids=list(range(N_CORES)))
    full = np.empty((1, HEADS * HID, H_IMG, OUT_DIM), np.float32)
    for h in range(HEADS):
        o = res.results[h]["out"].reshape(OUT_DIM, H_IMG, HID)
        full[0, HID * h : HID * (h + 1)] = o.transpose(2, 1, 0)
    return full


# revision 46
# speedup vs baseline: 1.6252x; 1.0114x over previous
"""Trainium2 Bass kernel for nn_MultiHeadSelfAttention2d (fp8 redesign).

Reference computation (B=1, C=64, H=32, W=128, HEADS=8, HIDDEN=16):
  q/k/v = 1x1 conv over channels (+bias), per-head attention over N=H*W=4096
  positions, softmax(q k^T / sqrt(16)), out = attn @ v, then a Linear over the
  W axis (W == HEADS*HIDDEN == 128) producing (1, 128, 32, 64).

Distribution: one head per NeuronCore -> 8 cores, fully independent.

Per-core dataflow:
  - proj:   V^T [m, d_aug] via X-stationary bf16 matmuls (bias + ones col
            folded into the augmented wva, all scaled x16), converted to
            fp8e4.  Q,K = W_aug x (bias row folded, x16) -> PSUM -> fp8e4
            in [d=16, (q|k), plane, n] layout where plane 1 is zeros.
  - S^T:    fp8 DoubleRow matmuls: lhsT = K[16,2,128] (plane1 = 0), rhs =
            Q[16,2,512] -> S^T*256 in PSUM at 0.5 cycles/col.
  - exp:    exp(u/1024) from PSUM -> fp8e4 P^T pair tiles [128, 2, n],
            split between ACT (hw exp) and DVE (cubic-poly custom op) by a
            greedy static schedule.
  - AV:     transposed accumulation: stationary = P^T pair [128,2,128],
            moving = V_aug pair [128,2,17] -> out [128(n=w), 17] per image
            row h, PSUM-accumulated over the 16 m-chunk pairs (DoubleRow).
            Output arrives already transposed; col 16 is the softmax
            denominator (V_aug ones column).
  - norm:   strided reciprocal of col 16, stride-0-broadcast tensor_tensor
            multiply -> OF [128(w), 32(h), 16(c)] bf16.
  - linear: out[o, (h,c)] = W_lin^T-stationary matmul over w + per-partition
            bias -> DMA out [64, 512] f32 (host transposes).
"""

import os
from contextlib import ExitStack

import ml_dtypes
import numpy as np

import concourse.bass as bass
import concourse.tile as tile
from concourse import bacc, mybir

# ---------------------------------------------------------------------------
# Problem constants (hardcoded per the task contract)
HEADS = 8
HID = 16
C_IN = 64
OUT_DIM = 64
H_IMG = 32
W_IMG = 128
N_TOK = H_IMG * W_IMG  # 4096
N_CORES = 8

BF16 = mybir.dt.bfloat16
F32 = mybir.dt.float32
FP8 = mybir.dt.float8e4
NPF8 = ml_dtypes.float8_e4m3

QK_SCALE = 16.0  # host-side scale on wq/wk/wv (and their biases)

# ---------------------------------------------------------------------------
# Custom DVE (vector engine) op: out = (((c3*u + c2)*u + c1)*u + 1)^2
# With c1=1/2048, c2=1/(2*2048^2), c3=1/(6*2048^3) this is exp(u/1024) to
# ~1e-5 rel for |u| < 220 (S*256 observed < 220).  Lets the Vector engine
# share softmax-exp work with the Scalar engine.
from concourse.dve_spec import Spec, Src0, C0, C1, C2, One, sq, lower
from concourse.dve_uop import DveOpSpec
from concourse import dve_ops
from concourse.dve_table_gen import dve_ver_for

EXP_C1 = 1.0 / 2048.0
EXP_C2 = 1.0 / (2.0 * 2048.0**2)
EXP_C3 = 1.0 / (6.0 * 2048.0**3)


def _exp_ref(in0, in1, c0, c1, c2):
    u = in0.astype(np.float32)
    q = ((np.float32(c2) * u + np.float32(c1)) * u + np.float32(c0)) * u + np.float32(
        1.0
    )
    return q * q


def _register_exp_op():
    name = "EXP_QTR_POLY_ANT"
    for op in dve_ops.OPS:
        if op.name == name:
            return op
    body = sq(((Src0 * C2 + C1) * Src0 + C0) * Src0 + One)
    spec = Spec(body=body, reference=_exp_ref)
    row = max(dve_ops._SUB_OPCODE_FOR_NAME.values()) + 1
    assert row < 0x20
    dve_ops._SUB_OPCODE_FOR_NAME[name] = row
    shas = {}
    for ver in ("v3", "v4"):
        try:
            uops = lower(spec, ver=ver)
            shas[ver] = DveOpSpec(name=name, opcode=row, uops=uops, rd1_en=False).sha(
                ver
            )
        except Exception:
            pass
    op = dve_ops.DveOp(name, spec, subdim=False, uops_sha=shas)
    dve_ops.OPS.append(op)
    dve_ops.CUSTOM_DVE_SPECS[name] = spec
    return op


EXP_OP = _register_exp_op()

# expm1 variant: out = ((c2*u + c1)*u + c0)*u  ~=  exp(u/1024) - 1
# (delta form keeps fp8 quantization error ~4x smaller near P=1)
EXPM1_C0 = 1.0 / 1024.0
EXPM1_C1 = 1.0 / 2097152.0
EXPM1_C2 = 4.0 / (3.0 * 2048.0**3)


def _expm1_ref(in0, in1, c0, c1, c2):
    u = in0.astype(np.float32)
    return ((np.float32(c2) * u + np.float32(c1)) * u + np.float32(c0)) * u


def _register_expm1_op():
    name = "EXPM1_CUBIC_ANT"
    for op in dve_ops.OPS:
        if op.name == name:
            return op
    body = ((Src0 * C2 + C1) * Src0 + C0) * Src0
    spec = Spec(body=body, reference=_expm1_ref)
    row = max(dve_ops._SUB_OPCODE_FOR_NAME.values()) + 1
    assert row < 0x20
    dve_ops._SUB_OPCODE_FOR_NAME[name] = row
    shas = {}
    for ver in ("v3", "v4"):
        try:
            uops = lower(spec, ver=ver)
            shas[ver] = DveOpSpec(name=name, opcode=row, uops=uops, rd1_en=False).sha(
                ver
            )
        except Exception:
            pass
    op = dve_ops.DveOp(name, spec, subdim=False, uops_sha=shas)
    dve_ops.OPS.append(op)
    dve_ops.CUSTOM_DVE_SPECS[name] = spec
    return op


EXPM1_OP = _register_expm1_op()

# m-chunks whose exp runs on DVE as delta=expm1 (odd chunks minus {1,3}):
# 14 of 32, balancing ACT 18*4*1038ns vs DVE 14*8*658ns.
D_CHUNKS = frozenset(mc for mc in range(1, 32, 2)) - {1, 3}


def _chunk_splits(n):
    """Split n columns into s-tile widths of 1024 (PSUM: 2 banks/tile)."""
    assert n % 1024 == 0
    return [(o, 1024) for o in range(0, n, 1024)]


# ---------------------------------------------------------------------------
def build_module(n_tok: int = N_TOK, act_bias_ns: float = -1000.0, pt_bufs: int = 5, pend_max: int = 32):
    """Builds (and bacc-compiles) the per-core Bass module."""
    assert n_tok % 1024 == 0
    mch = n_tok // 128  # m chunks (32)
    npair = mch // 2  # DoubleRow pairs (16)
    nh = n_tok // W_IMG  # image rows (32)
    f_tot = nh * HID  # (h,c) columns of the final output (512)

    nc = bacc.Bacc()

    xin = nc.dram_tensor("xin", [C_IN + 1, n_tok], BF16, kind="ExternalInput")
    wpk = nc.dram_tensor("wpk", [C_IN + 1, 2 * HID + HID + 1], BF16, kind="ExternalInput")
    ztil = nc.dram_tensor("ztil_v6", [HID, n_tok], FP8, kind="ExternalInput")
    wlt = nc.dram_tensor("wlt", [W_IMG, OUT_DIM], BF16, kind="ExternalInput")
    blb = nc.dram_tensor("blb", [OUT_DIM, 1], F32, kind="ExternalInput")
    out = nc.dram_tensor("out", [OUT_DIM, f_tot], F32, kind="ExternalOutput")

    # --- engine assignment: time-balanced greedy over per-engine rings -----
    ACT_RATE, ACT_INIT = 1.0 / 1.2, 185.0
    DVE_RATE, DVE_INIT = 1.0 / 0.96, 125.0
    eng_t = {"A": 1283.0 + act_bias_ns, "D": 0.0}

    def charge_engine(eng, width):
        if eng == "A":
            eng_t["A"] += width * ACT_RATE + ACT_INIT
        else:
            eng_t["D"] += width * DVE_RATE + DVE_INIT

    def pick_engine(width_a, width_d=None):
        """Pick engine for an op of width_a on ACT / width_d on DVE."""
        if width_d is None:
            width_d = width_a
        ca = width_a * ACT_RATE + ACT_INIT
        cd = width_d * DVE_RATE + DVE_INIT
        if eng_t["A"] + ca <= eng_t["D"] + cd:
            eng_t["A"] += ca
            return "A"
        eng_t["D"] += cd
        return "D"

    with tile.TileContext(nc) as tc, ExitStack() as ctx:
        const = ctx.enter_context(tc.tile_pool(name="const", bufs=1))

        WPK = const.tile([C_IN + 1, 2 * HID + HID + 1], BF16)
        nc.sync.dma_start(WPK[:], wpk.ap())
        WQA = WPK[:, 0:HID]
        WKA = WPK[:, HID : 2 * HID]
        WVA = WPK[:, 2 * HID : 3 * HID + 1]

        XB = const.tile([C_IN + 1, n_tok], BF16)
        QK8 = const.tile([HID, 2, 2, n_tok], FP8)
        WL = const.tile([W_IMG, OUT_DIM], BF16)
        BLB = const.tile([OUT_DIM, 1], F32)
        for q0, q1 in [(0, 512), (512, 1024), (1024, 2048), (2048, 3072), (3072, 4096)]:
            q0 = q0 * n_tok // 4096
            q1 = q1 * n_tok // 4096
            nc.sync.dma_start(XB[:, q0:q1], xin.ap()[:, q0:q1])
        nc.sync.dma_start(QK8[:, 0, 1, :], ztil.ap())
        nc.sync.dma_start(QK8[:, 1, 1, :], ztil.ap())
        nc.sync.dma_start(WL[:], wlt.ap())
        nc.sync.dma_start(BLB[:], blb.ap())

        VA = const.tile([128, npair, 2, HID + 1], FP8)
        VASD = const.tile([128, 4, HID + 1], F32)  # per-group D-chunk V sums
        VSD = const.tile([128, HID + 1], F32)  # sum over all D-chunk m of V_aug
        VSDH = const.tile([128, HID + 1], BF16)
        VSDL = const.tile([128, HID + 1], BF16)
        ONESB = const.tile([128, 128], BF16)
        RC = const.tile([128, nh], F32)
        OF = const.tile([128, nh, HID], BF16)
        RES = const.tile([OUT_DIM, f_tot], F32)

        # ---- attention: S^T (fp8 DoubleRow) -> exp -> AV (transposed) ------
        with tc.tile_pool(name="s_pool", bufs=1, space="PSUM") as s_pool, \
             tc.tile_pool(name="av_pool", bufs=1, space="PSUM") as av_pool, \
             tc.tile_pool(name="pt_pool", bufs=pt_bufs) as pt_pool:
            # one [128, 17] accumulator per image row h, packed 16 per bank
            av = av_pool.tile([128, nh, 32], F32, tag="av", name="av")

            # ---- projections, emitted through the same s-tile rings --------
            for sc in range(n_tok // 512):
                ps = s_pool.tile([128, 1024], F32, tag="sa", bufs=2, name="sa")
                cs = slice(512 * sc, 512 * sc + 512)
                nc.tensor.matmul(ps[0:HID, 0:512], lhsT=WQA, rhs=XB[:, cs])
                nc.tensor.matmul(ps[0:HID, 512:1024], lhsT=WKA, rhs=XB[:, cs])
                src_ap = ps[0:HID, :].rearrange("p (a b) -> p a b", b=512)
                dst = QK8[:, :, 0, cs]
                if pick_engine(1024) == "A":
                    nc.scalar.activation(
                        dst, src_ap, mybir.ActivationFunctionType.Copy
                    )
                else:
                    nc.vector.tensor_copy(dst, src_ap)
                if sc % 2 == 1:
                    g = sc // 2
                    psv = s_pool.tile([128, 512], F32, tag="sd", bufs=2, name="sd")
                    for vj in range(8):
                        mc = 8 * g + vj
                        nc.tensor.matmul(
                            psv[:, 64 * vj : 64 * vj + HID + 1],
                            lhsT=XB[:, 128 * mc : 128 * mc + 128],
                            rhs=WVA,
                        )
                    vsrc = psv[:].rearrange("p (a b) -> p a b", b=64)[:, :, 0 : HID + 1]
                    vdst = VA[:, 4 * g : 4 * g + 4, :, :]
                    if pick_engine(136) == "A":
                        nc.scalar.activation(
                            vdst, vsrc, mybir.ActivationFunctionType.Copy
                        )
                    else:
                        nc.vector.tensor_copy(vdst, vsrc)
                    # f32 sums of this group's D-chunk V columns (delta
                    # add-back); D positions: g0 -> {5,7}, else odd
                    vt = psv[:].rearrange("p (a b) -> p b a", b=64)
                    dsl = (
                        vt[:, 0 : HID + 1, 5:8:2]
                        if g == 0
                        else vt[:, 0 : HID + 1, 1:8:2]
                    )
                    nc.vector.tensor_reduce(
                        VASD[:, g, :], dsl, axis=mybir.AxisListType.X,
                        op=mybir.AluOpType.add,
                    )
            nc.vector.tensor_reduce(
                VSD[:], VASD[:].rearrange("p g d -> p d g"),
                axis=mybir.AxisListType.X, op=mybir.AluOpType.add,
            )
            nc.vector.tensor_copy(VSDH[:], VSD[:])
            nc.vector.tensor_tensor(
                VSDL[:], VSD[:], VSDH[:], op=mybir.AluOpType.subtract
            )
            nc.gpsimd.memset(ONESB[:], 1.0)

            def av_mm(j, pt, h):
                nc.tensor.matmul(
                    av[:, h, 0 : HID + 1],
                    lhsT=pt[:, :, 128 * h : 128 * h + 128],
                    rhs=VA[:, j, :, :],
                    perf_mode=mybir.MatmulPerfMode.DoubleRow,
                    start=False,
                    stop=(j == npair - 1),
                    skip_group_check=True,
                )

            # seed each accumulator with sum_{m in D-chunks} V_aug[m, :]
            # (bf16 hi+lo ones-matmuls; delta-form chunks contribute P-1)
            for h in range(nh):
                # start=True only on the first write of each PSUM bank: a
                # start re-zeroes the whole bank's accumulation group, so
                # later regions must join with start=False
                nc.tensor.matmul(
                    av[:, h, 0 : HID + 1],
                    lhsT=ONESB[:],
                    rhs=VSDH[:],
                    start=(h % 16 == 0),
                    stop=False,
                    skip_group_check=True,
                )
                nc.tensor.matmul(
                    av[:, h, 0 : HID + 1],
                    lhsT=ONESB[:],
                    rhs=VSDL[:],
                    start=False,
                    stop=False,
                    skip_group_check=True,
                )

            def emit_slot(j, t, col, w, eng, pt):
                mc = 2 * j + t
                if eng == "A":
                    s = s_pool.tile([128, 1024], F32, tag="sa", bufs=2, name="sa")
                else:
                    s = s_pool.tile([128, 512], F32, tag="sd", bufs=2, name="sd")
                for sub in range(0, w, 512):
                    nc.tensor.matmul(
                        s[:, sub : sub + 512],
                        lhsT=QK8[:, 1, :, 128 * mc : 128 * mc + 128],
                        rhs=QK8[:, 0, :, col + sub : col + sub + 512],
                        perf_mode=mybir.MatmulPerfMode.DoubleRow,
                    )
                dstp = pt[:, t, col : col + w]
                if eng == "A":
                    nc.scalar.activation(
                        dstp,
                        s[:, 0:w],
                        mybir.ActivationFunctionType.Exp,
                        scale=1.0 / 1024.0,
                    )
                else:
                    nc.vector._custom_dve(
                        EXPM1_OP,
                        out=dstp,
                        in0=s[:, 0:w],
                        s0=EXPM1_C0,
                        s1=EXPM1_C1,
                        imm2=EXPM1_C2,
                    )

            # build per-engine slot streams (chunk -> engine fixed by
            # D_CHUNKS), then merge by projected engine finish time so the
            # PE feeds both rings concurrently
            streams = {"A": [], "D": []}
            slots_left = [0] * npair
            for j in range(npair):
                for t in range(2):
                    mc = 2 * j + t
                    eng = "D" if mc in D_CHUNKS else "A"
                    w = 1024 if eng == "A" else 512
                    for col in range(0, n_tok, w):
                        streams[eng].append((j, t, col, w))
                        slots_left[j] += 1

            pt_tiles = {}

            def get_pt(j):
                if j not in pt_tiles:
                    pt_tiles[j] = pt_pool.tile(
                        [128, 2, n_tok], FP8, tag="pt", name="pt"
                    )
                return pt_tiles[j]

            pend = []
            ii = {"A": 0, "D": 0}
            lastj = npair - 1
            prog = [0, 0]  # column progress of the last pair's two planes
            next_h = 0  # next last-pair AV row to emit
            while ii["A"] < len(streams["A"]) or ii["D"] < len(streams["D"]):
                if ii["A"] >= len(streams["A"]):
                    eng = "D"
                elif ii["D"] >= len(streams["D"]):
                    eng = "A"
                else:
                    wa = streams["A"][ii["A"]][3]
                    wd = streams["D"][ii["D"]][3]
                    ca = eng_t["A"] + wa * ACT_RATE + ACT_INIT
                    cd = eng_t["D"] + wd * DVE_RATE + DVE_INIT
                    eng = "A" if ca <= cd else "D"
                j, t, col, w = streams[eng][ii[eng]]
                ii[eng] += 1
                charge_engine(eng, w)
                emit_slot(j, t, col, w, eng, get_pt(j))
                slots_left[j] -= 1
                if slots_left[j] == 0 and j != lastj:
                    pend.extend((j, pt_tiles[j], h) for h in range(nh))
                if j == lastj:
                    # progressive tail: drain earlier pairs, then emit each
                    # last-pair AV stop as soon as both planes cover its row
                    for args in pend:
                        av_mm(*args)
                    pend = []
                    prog[t] = col + w
                    lim = min(prog) // W_IMG
                    while next_h < lim:
                        av_mm(lastj, get_pt(lastj), next_h)
                        next_h += 1
                else:
                    while len(pend) > pend_max:
                        av_mm(*pend.pop(0))
            while next_h < nh:
                av_mm(lastj, get_pt(lastj), next_h)
                next_h += 1
            for args in pend:
                av_mm(*args)

            # ---- normalize: reciprocal of denominators + broadcast mult ----
            for b in range(nh // 16):
                hs = slice(16 * b, 16 * b + 16)
                nc.vector.reciprocal(RC[:, hs], av[:, hs, HID : HID + 1])
                nc.vector.tensor_tensor(
                    OF[:, hs, :],
                    av[:, hs, 0:HID],
                    RC[:, hs].unsqueeze(2).broadcast_to([128, 16, HID]),
                    op=mybir.AluOpType.mult,
                )

        # ---- final linear ---------------------------------------------------
        with tc.tile_pool(name="tail_ps", bufs=1, space="PSUM") as tail_ps:
            for b in range(nh // 16):
                fs = slice(256 * b, 256 * b + 256)
                psf = tail_ps.tile([OUT_DIM, 256], F32, tag="f", bufs=2, name="psf")
                nc.tensor.matmul(
                    psf[:], lhsT=WL[:], rhs=OF[:, 16 * b : 16 * b + 16, :]
                )
                nc.scalar.activation(
                    RES[:, fs], psf[:], mybir.ActivationFunctionType.Identity,
                    bias=BLB[:],
                )
                nc.sync.dma_start(out.ap()[:, fs], RES[:, fs])

    nc.compile()
    return nc


# ---------------------------------------------------------------------------
def make_core_inputs(x, wq, bq, wk, bk, wv, bv, w_lin, b_lin, n_tok=N_TOK):
    """Host-side prep: full inputs -> list of 8 per-core input dicts."""
    X = np.asarray(x, np.float32).reshape(C_IN, -1)[:, :n_tok]
    xa = np.ones((C_IN + 1, n_tok), np.float32)
    xa[:C_IN] = X
    xin = xa.astype(ml_dtypes.bfloat16)
    wlt = np.ascontiguousarray(np.asarray(w_lin, np.float32).T).astype(
        ml_dtypes.bfloat16
    )
    blb = np.asarray(b_lin, np.float32).reshape(OUT_DIM, 1)
    ztil = np.zeros((HID, n_tok), NPF8)

    s = QK_SCALE
    maps = []
    for h in range(HEADS):
        sl = slice(HID * h, HID * (h + 1))

        def aug(w, b, ones_col=False):
            d = HID + 1 if ones_col else HID
            m = np.zeros((C_IN + 1, d), np.float32)
            m[0:C_IN, 0:HID] = s * np.asarray(w, np.float32)[sl].T
            m[C_IN, 0:HID] = s * np.asarray(b, np.float32)[sl]
            if ones_col:
                m[C_IN, HID] = s
            return m.astype(ml_dtypes.bfloat16)

        wpk = np.concatenate(
            [
                np.asarray(aug(wq, bq), np.float32),
                np.asarray(aug(wk, bk), np.float32),
                np.asarray(aug(wv, bv, ones_col=True), np.float32),
            ],
            axis=1,
        ).astype(ml_dtypes.bfloat16)
        maps.append(
            {
                "xin": xin,
                "wpk": wpk,
                "ztil_v6": ztil,
                "wlt": wlt,
                "blb": blb,
            }
        )
    return maps


_MODULE_CACHE = {}


def _get_module(**kw):
    key = tuple(sorted(kw.items()))
    if key not in _MODULE_CACHE:
        _MODULE_CACHE[key] = build_module(**kw)
    return _MODULE_CACHE[key]


def kernel(x, wq, bq, wk, bk, wv, bv, w_lin, b_lin):
    from concourse.bass_utils import run_bass_kernel_spmd

    nc = _get_module()
    in_maps = make_core_inputs(x, wq, bq, wk, bk, wv, bv, w_lin, b_lin)
    res = run_bass_kernel_spmd(nc, in_maps, core_ids=list(range(N_CORES)))
    full = np.empty((1, HEADS * HID, H_IMG, OUT_DIM), np.float32)
    for h in range(HEADS):
        o = res.results[h]["out"].reshape(OUT_DIM, H_IMG, HID)
        full[0, HID * h : HID * (h + 1)] = o.transpose(2, 1, 0)
    return full


# revision 47
# speedup vs baseline: 1.6252x; 1.0000x over previous
"""Trainium2 Bass kernel for nn_MultiHeadSelfAttention2d (fp8 redesign).

Reference computation (B=1, C=64, H=32, W=128, HEADS=8, HIDDEN=16):
  q/k/v = 1x1 conv over channels (+bias), per-head attention over N=H*W=4096
  positions, softmax(q k^T / sqrt(16)), out = attn @ v, then a Linear over the
  W axis (W == HEADS*HIDDEN == 128) producing (1, 128, 32, 64).

Distribution: one head per NeuronCore -> 8 cores, fully independent.

Per-core dataflow:
  - proj:   V^T [m, d_aug] via X-stationary bf16 matmuls (bias + ones col
            folded into the augmented wva, all scaled x16), converted to
            fp8e4.  Q,K = W_aug x (bias row folded, x16) -> PSUM -> fp8e4
            in [d=16, (q|k), plane, n] layout where plane 1 is zeros.
  - S^T:    fp8 DoubleRow matmuls: lhsT = K[16,2,128] (plane1 = 0), rhs =
            Q[16,2,512] -> S^T*256 in PSUM at 0.5 cycles/col.
  - exp:    exp(u/1024) from PSUM -> fp8e4 P^T pair tiles [128, 2, n],
            split between ACT (hw exp) and DVE (cubic-poly custom op) by a
            greedy static schedule.
  - AV:     transposed accumulation: stationary = P^T pair [128,2,128],
            moving = V_aug pair [128,2,17] -> out [128(n=w), 17] per image
            row h, PSUM-accumulated over the 16 m-chunk pairs (DoubleRow).
            Output arrives already transposed; col 16 is the softmax
            denominator (V_aug ones column).
  - norm:   strided reciprocal of col 16, stride-0-broadcast tensor_tensor
            multiply -> OF [128(w), 32(h), 16(c)] bf16.
  - linear: out[o, (h,c)] = W_lin^T-stationary matmul over w + per-partition
            bias -> DMA out [64, 512] f32 (host transposes).
"""

import os
from contextlib import ExitStack

import ml_dtypes
import numpy as np

import concourse.bass as bass
import concourse.tile as tile
from concourse import bacc, mybir

# ---------------------------------------------------------------------------
# Problem constants (hardcoded per the task contract)
HEADS = 8
HID = 16
C_IN = 64
OUT_DIM = 64
H_IMG = 32
W_IMG = 128
N_TOK = H_IMG * W_IMG  # 4096
N_CORES = 8

BF16 = mybir.dt.bfloat16
F32 = mybir.dt.float32
FP8 = mybir.dt.float8e4
NPF8 = ml_dtypes.float8_e4m3

QK_SCALE = 16.0  # host-side scale on wq/wk/wv (and their biases)

# ---------------------------------------------------------------------------
# Custom DVE (vector engine) op: out = (((c3*u + c2)*u + c1)*u + 1)^2
# With c1=1/2048, c2=1/(2*2048^2), c3=1/(6*2048^3) this is exp(u/1024) to
# ~1e-5 rel for |u| < 220 (S*256 observed < 220).  Lets the Vector engine
# share softmax-exp work with the Scalar engine.
from concourse.dve_spec import Spec, Src0, C0, C1, C2, One, sq, lower
from concourse.dve_uop import DveOpSpec
from concourse import dve_ops
from concourse.dve_table_gen import dve_ver_for

EXP_C1 = 1.0 / 2048.0
EXP_C2 = 1.0 / (2.0 * 2048.0**2)
EXP_C3 = 1.0 / (6.0 * 2048.0**3)


def _exp_ref(in0, in1, c0, c1, c2):
    u = in0.astype(np.float32)
    q = ((np.float32(c2) * u + np.float32(c1)) * u + np.float32(c0)) * u + np.float32(
        1.0
    )
    return q * q


def _register_exp_op():
    name = "EXP_QTR_POLY_ANT"
    for op in dve_ops.OPS:
        if op.name == name:
            return op
    body = sq(((Src0 * C2 + C1) * Src0 + C0) * Src0 + One)
    spec = Spec(body=body, reference=_exp_ref)
    row = max(dve_ops._SUB_OPCODE_FOR_NAME.values()) + 1
    assert row < 0x20
    dve_ops._SUB_OPCODE_FOR_NAME[name] = row
    shas = {}
    for ver in ("v3", "v4"):
        try:
            uops = lower(spec, ver=ver)
            shas[ver] = DveOpSpec(name=name, opcode=row, uops=uops, rd1_en=False).sha(
                ver
            )
        except Exception:
            pass
    op = dve_ops.DveOp(name, spec, subdim=False, uops_sha=shas)
    dve_ops.OPS.append(op)
    dve_ops.CUSTOM_DVE_SPECS[name] = spec
    return op


EXP_OP = _register_exp_op()

# expm1 variant: out = ((c2*u + c1)*u + c0)*u  ~=  exp(u/1024) - 1
# (delta form keeps fp8 quantization error ~4x smaller near P=1)
EXPM1_C0 = 1.0 / 1024.0
EXPM1_C1 = 1.0 / 2097152.0
EXPM1_C2 = 4.0 / (3.0 * 2048.0**3)


def _expm1_ref(in0, in1, c0, c1, c2):
    u = in0.astype(np.float32)
    return ((np.float32(c2) * u + np.float32(c1)) * u + np.float32(c0)) * u


def _register_expm1_op():
    name = "EXPM1_CUBIC_ANT"
    for op in dve_ops.OPS:
        if op.name == name:
            return op
    body = ((Src0 * C2 + C1) * Src0 + C0) * Src0
    spec = Spec(body=body, reference=_expm1_ref)
    row = max(dve_ops._SUB_OPCODE_FOR_NAME.values()) + 1
    assert row < 0x20
    dve_ops._SUB_OPCODE_FOR_NAME[name] = row
    shas = {}
    for ver in ("v3", "v4"):
        try:
            uops = lower(spec, ver=ver)
            shas[ver] = DveOpSpec(name=name, opcode=row, uops=uops, rd1_en=False).sha(
                ver
            )
        except Exception:
            pass
    op = dve_ops.DveOp(name, spec, subdim=False, uops_sha=shas)
    dve_ops.OPS.append(op)
    dve_ops.CUSTOM_DVE_SPECS[name] = spec
    return op


EXPM1_OP = _register_expm1_op()

# m-chunks whose exp runs on DVE as delta=expm1 (odd chunks minus {1,3}):
# 14 of 32, balancing ACT 18*4*1038ns vs DVE 14*8*658ns.
D_CHUNKS = frozenset(mc for mc in range(1, 32, 2)) - {1, 3}


def _chunk_splits(n):
    """Split n columns into s-tile widths of 1024 (PSUM: 2 banks/tile)."""
    assert n % 1024 == 0
    return [(o, 1024) for o in range(0, n, 1024)]


# ---------------------------------------------------------------------------
def build_module(n_tok: int = N_TOK, act_bias_ns: float = -1100.0, pt_bufs: int = 5, pend_max: int = 32):
    """Builds (and bacc-compiles) the per-core Bass module."""
    assert n_tok % 1024 == 0
    mch = n_tok // 128  # m chunks (32)
    npair = mch // 2  # DoubleRow pairs (16)
    nh = n_tok // W_IMG  # image rows (32)
    f_tot = nh * HID  # (h,c) columns of the final output (512)

    nc = bacc.Bacc()

    xin = nc.dram_tensor("xin", [C_IN + 1, n_tok], BF16, kind="ExternalInput")
    wpk = nc.dram_tensor("wpk", [C_IN + 1, 2 * HID + HID + 1], BF16, kind="ExternalInput")
    ztil = nc.dram_tensor("ztil_v6", [HID, n_tok], FP8, kind="ExternalInput")
    wlt = nc.dram_tensor("wlt", [W_IMG, OUT_DIM], BF16, kind="ExternalInput")
    blb = nc.dram_tensor("blb", [OUT_DIM, 1], F32, kind="ExternalInput")
    out = nc.dram_tensor("out", [OUT_DIM, f_tot], F32, kind="ExternalOutput")

    # --- engine assignment: time-balanced greedy over per-engine rings -----
    ACT_RATE, ACT_INIT = 1.0 / 1.2, 185.0
    DVE_RATE, DVE_INIT = 1.0 / 0.96, 125.0
    eng_t = {"A": 1283.0 + act_bias_ns, "D": 0.0}

    def charge_engine(eng, width):
        if eng == "A":
            eng_t["A"] += width * ACT_RATE + ACT_INIT
        else:
            eng_t["D"] += width * DVE_RATE + DVE_INIT

    def pick_engine(width_a, width_d=None):
        """Pick engine for an op of width_a on ACT / width_d on DVE."""
        if width_d is None:
            width_d = width_a
        ca = width_a * ACT_RATE + ACT_INIT
        cd = width_d * DVE_RATE + DVE_INIT
        if eng_t["A"] + ca <= eng_t["D"] + cd:
            eng_t["A"] += ca
            return "A"
        eng_t["D"] += cd
        return "D"

    with tile.TileContext(nc) as tc, ExitStack() as ctx:
        const = ctx.enter_context(tc.tile_pool(name="const", bufs=1))

        WPK = const.tile([C_IN + 1, 2 * HID + HID + 1], BF16)
        nc.sync.dma_start(WPK[:], wpk.ap())
        WQA = WPK[:, 0:HID]
        WKA = WPK[:, HID : 2 * HID]
        WVA = WPK[:, 2 * HID : 3 * HID + 1]

        XB = const.tile([C_IN + 1, n_tok], BF16)
        QK8 = const.tile([HID, 2, 2, n_tok], FP8)
        WL = const.tile([W_IMG, OUT_DIM], BF16)
        BLB = const.tile([OUT_DIM, 1], F32)
        for q0, q1 in [(0, 512), (512, 1024), (1024, 2048), (2048, 3072), (3072, 4096)]:
            q0 = q0 * n_tok // 4096
            q1 = q1 * n_tok // 4096
            nc.sync.dma_start(XB[:, q0:q1], xin.ap()[:, q0:q1])
        nc.sync.dma_start(QK8[:, 0, 1, :], ztil.ap())
        nc.sync.dma_start(QK8[:, 1, 1, :], ztil.ap())
        nc.sync.dma_start(WL[:], wlt.ap())
        nc.sync.dma_start(BLB[:], blb.ap())

        VA = const.tile([128, npair, 2, HID + 1], FP8)
        VASD = const.tile([128, 4, HID + 1], F32)  # per-group D-chunk V sums
        VSD = const.tile([128, HID + 1], F32)  # sum over all D-chunk m of V_aug
        VSDH = const.tile([128, HID + 1], BF16)
        VSDL = const.tile([128, HID + 1], BF16)
        ONESB = const.tile([128, 128], BF16)
        RC = const.tile([128, nh], F32)
        OF = const.tile([128, nh, HID], BF16)
        RES = const.tile([OUT_DIM, f_tot], F32)

        # ---- attention: S^T (fp8 DoubleRow) -> exp -> AV (transposed) ------
        with tc.tile_pool(name="s_pool", bufs=1, space="PSUM") as s_pool, \
             tc.tile_pool(name="av_pool", bufs=1, space="PSUM") as av_pool, \
             tc.tile_pool(name="pt_pool", bufs=pt_bufs) as pt_pool:
            # one [128, 17] accumulator per image row h, packed 16 per bank
            av = av_pool.tile([128, nh, 32], F32, tag="av", name="av")

            # ---- projections, emitted through the same s-tile rings --------
            for sc in range(n_tok // 512):
                ps = s_pool.tile([128, 1024], F32, tag="sa", bufs=2, name="sa")
                cs = slice(512 * sc, 512 * sc + 512)
                nc.tensor.matmul(ps[0:HID, 0:512], lhsT=WQA, rhs=XB[:, cs])
                nc.tensor.matmul(ps[0:HID, 512:1024], lhsT=WKA, rhs=XB[:, cs])
                src_ap = ps[0:HID, :].rearrange("p (a b) -> p a b", b=512)
                dst = QK8[:, :, 0, cs]
                if pick_engine(1024) == "A":
                    nc.scalar.activation(
                        dst, src_ap, mybir.ActivationFunctionType.Copy
                    )
                else:
                    nc.vector.tensor_copy(dst, src_ap)
                if sc % 2 == 1:
                    g = sc // 2
                    psv = s_pool.tile([128, 512], F32, tag="sd", bufs=2, name="sd")
                    for vj in range(8):
                        mc = 8 * g + vj
                        nc.tensor.matmul(
                            psv[:, 64 * vj : 64 * vj + HID + 1],
                            lhsT=XB[:, 128 * mc : 128 * mc + 128],
                            rhs=WVA,
                        )
                    vsrc = psv[:].rearrange("p (a b) -> p a b", b=64)[:, :, 0 : HID + 1]
                    vdst = VA[:, 4 * g : 4 * g + 4, :, :]
                    if pick_engine(136) == "A":
                        nc.scalar.activation(
                            vdst, vsrc, mybir.ActivationFunctionType.Copy
                        )
                    else:
                        nc.vector.tensor_copy(vdst, vsrc)
                    # f32 sums of this group's D-chunk V columns (delta
                    # add-back); D positions: g0 -> {5,7}, else odd
                    vt = psv[:].rearrange("p (a b) -> p b a", b=64)
                    dsl = (
                        vt[:, 0 : HID + 1, 5:8:2]
                        if g == 0
                        else vt[:, 0 : HID + 1, 1:8:2]
                    )
                    nc.vector.tensor_reduce(
                        VASD[:, g, :], dsl, axis=mybir.AxisListType.X,
                        op=mybir.AluOpType.add,
                    )
            nc.vector.tensor_reduce(
                VSD[:], VASD[:].rearrange("p g d -> p d g"),
                axis=mybir.AxisListType.X, op=mybir.AluOpType.add,
            )
            nc.vector.tensor_copy(VSDH[:], VSD[:])
            nc.vector.tensor_tensor(
                VSDL[:], VSD[:], VSDH[:], op=mybir.AluOpType.subtract
            )
            nc.gpsimd.memset(ONESB[:], 1.0)

            def av_mm(j, pt, h):
                nc.tensor.matmul(
                    av[:, h, 0 : HID + 1],
                    lhsT=pt[:, :, 128 * h : 128 * h + 128],
                    rhs=VA[:, j, :, :],
                    perf_mode=mybir.MatmulPerfMode.DoubleRow,
                    start=False,
                    stop=(j == npair - 1),
                    skip_group_check=True,
                )

            # seed each accumulator with sum_{m in D-chunks} V_aug[m, :]
            # (bf16 hi+lo ones-matmuls; delta-form chunks contribute P-1)
            for h in range(nh):
                # start=True only on the first write of each PSUM bank: a
                # start re-zeroes the whole bank's accumulation group, so
                # later regions must join with start=False
                nc.tensor.matmul(
                    av[:, h, 0 : HID + 1],
                    lhsT=ONESB[:],
                    rhs=VSDH[:],
                    start=(h % 16 == 0),
                    stop=False,
                    skip_group_check=True,
                )
                nc.tensor.matmul(
                    av[:, h, 0 : HID + 1],
                    lhsT=ONESB[:],
                    rhs=VSDL[:],
                    start=False,
                    stop=False,
                    skip_group_check=True,
                )

            def emit_slot(j, t, col, w, eng, pt):
                mc = 2 * j + t
                if eng == "A":
                    s = s_pool.tile([128, 1024], F32, tag="sa", bufs=2, name="sa")
                else:
                    s = s_pool.tile([128, 512], F32, tag="sd", bufs=2, name="sd")
                for sub in range(0, w, 512):
                    nc.tensor.matmul(
                        s[:, sub : sub + 512],
                        lhsT=QK8[:, 1, :, 128 * mc : 128 * mc + 128],
                        rhs=QK8[:, 0, :, col + sub : col + sub + 512],
                        perf_mode=mybir.MatmulPerfMode.DoubleRow,
                    )
                dstp = pt[:, t, col : col + w]
                if eng == "A":
                    nc.scalar.activation(
                        dstp,
                        s[:, 0:w],
                        mybir.ActivationFunctionType.Exp,
                        scale=1.0 / 1024.0,
                    )
                else:
                    nc.vector._custom_dve(
                        EXPM1_OP,
                        out=dstp,
                        in0=s[:, 0:w],
                        s0=EXPM1_C0,
                        s1=EXPM1_C1,
                        imm2=EXPM1_C2,
                    )

            # build per-engine slot streams (chunk -> engine fixed by
            # D_CHUNKS), then merge by projected engine finish time so the
            # PE feeds both rings concurrently
            streams = {"A": [], "D": []}
            slots_left = [0] * npair
            for j in range(npair):
                for t in range(2):
                    mc = 2 * j + t
                    eng = "D" if mc in D_CHUNKS else "A"
                    w = 1024 if eng == "A" else 512
                    for col in range(0, n_tok, w):
                        streams[eng].append((j, t, col, w))
                        slots_left[j] += 1

            pt_tiles = {}

            def get_pt(j):
                if j not in pt_tiles:
                    pt_tiles[j] = pt_pool.tile(
                        [128, 2, n_tok], FP8, tag="pt", name="pt"
                    )
                return pt_tiles[j]

            pend = []
            ii = {"A": 0, "D": 0}
            lastj = npair - 1
            prog = [0, 0]  # column progress of the last pair's two planes
            next_h = 0  # next last-pair AV row to emit
            while ii["A"] < len(streams["A"]) or ii["D"] < len(streams["D"]):
                if ii["A"] >= len(streams["A"]):
                    eng = "D"
                elif ii["D"] >= len(streams["D"]):
                    eng = "A"
                else:
                    wa = streams["A"][ii["A"]][3]
                    wd = streams["D"][ii["D"]][3]
                    ca = eng_t["A"] + wa * ACT_RATE + ACT_INIT
                    cd = eng_t["D"] + wd * DVE_RATE + DVE_INIT
                    eng = "A" if ca <= cd else "D"
                j, t, col, w = streams[eng][ii[eng]]
                ii[eng] += 1
                charge_engine(eng, w)
                emit_slot(j, t, col, w, eng, get_pt(j))
                slots_left[j] -= 1
                if slots_left[j] == 0 and j != lastj:
                    pend.extend((j, pt_tiles[j], h) for h in range(nh))
                if j == lastj:
                    # progressive tail: drain earlier pairs, then emit each
                    # last-pair AV stop as soon as both planes cover its row
                    for args in pend:
                        av_mm(*args)
                    pend = []
                    prog[t] = col + w
                    lim = min(prog) // W_IMG
                    while next_h < lim:
                        av_mm(lastj, get_pt(lastj), next_h)
                        next_h += 1
                else:
                    while len(pend) > pend_max:
                        av_mm(*pend.pop(0))
            while next_h < nh:
                av_mm(lastj, get_pt(lastj), next_h)
                next_h += 1
            for args in pend:
                av_mm(*args)

            # ---- normalize: reciprocal of denominators + broadcast mult ----
            for b in range(nh // 16):
                hs = slice(16 * b, 16 * b + 16)
                nc.vector.reciprocal(RC[:, hs], av[:, hs, HID : HID + 1])
                nc.vector.tensor_tensor(
                    OF[:, hs, :],
                    av[:, hs, 0:HID],
                    RC[:, hs].unsqueeze(2).broadcast_to([128, 16, HID]),
                    op=mybir.AluOpType.mult,
                )

        # ---- final linear ---------------------------------------------------
        with tc.tile_pool(name="tail_ps", bufs=1, space="PSUM") as tail_ps:
            for b in range(nh // 16):
                fs = slice(256 * b, 256 * b + 256)
                psf = tail_ps.tile([OUT_DIM, 256], F32, tag="f", bufs=2, name="psf")
                nc.tensor.matmul(
                    psf[:], lhsT=WL[:], rhs=OF[:, 16 * b : 16 * b + 16, :]
                )
                nc.scalar.activation(
                    RES[:, fs], psf[:], mybir.ActivationFunctionType.Identity,
                    bias=BLB[:],
                )
                nc.sync.dma_start(out.ap()[:, fs], RES[:, fs])

    nc.compile()
    return nc


# ---------------------------------------------------------------------------
def make_core_inputs(x, wq, bq, wk, bk, wv, bv, w_lin, b_lin, n_tok=N_TOK):
    """Host-side prep: full inputs -> list of 8 per-core input dicts."""
    X = np.asarray(x, np.float32).reshape(C_IN, -1)[:, :n_tok]
    xa = np.ones((C_IN + 1, n_tok), np.float32)
    xa[:C_IN] = X
    xin = xa.astype(ml_dtypes.bfloat16)
    wlt = np.ascontiguousarray(np.asarray(w_lin, np.float32).T).astype(
        ml_dtypes.bfloat16
    )
    blb = np.asarray(b_lin, np.float32).reshape(OUT_DIM, 1)
    ztil = np.zeros((HID, n_tok), NPF8)

    s = QK_SCALE
    maps = []
    for h in range(HEADS):
        sl = slice(HID * h, HID * (h + 1))

        def aug(w, b, ones_col=False):
            d = HID + 1 if ones_col else HID
            m = np.zeros((C_IN + 1, d), np.float32)
            m[0:C_IN, 0:HID] = s * np.asarray(w, np.float32)[sl].T
            m[C_IN, 0:HID] = s * np.asarray(b, np.float32)[sl]
            if ones_col:
                m[C_IN, HID] = s
            return m.astype(ml_dtypes.bfloat16)

        wpk = np.concatenate(
            [
                np.asarray(aug(wq, bq), np.float32),
                np.asarray(aug(wk, bk), np.float32),
                np.asarray(aug(wv, bv, ones_col=True), np.float32),
            ],
            axis=1,
        ).astype(ml_dtypes.bfloat16)
        maps.append(
            {
                "xin": xin,
                "wpk": wpk,
                "ztil_v6": ztil,
                "wlt": wlt,
                "blb": blb,
            }
        )
    return maps


_MODULE_CACHE = {}


def _get_module(**kw):
    key = tuple(sorted(kw.items()))
    if key not in _MODULE_CACHE:
        _MODULE_CACHE[key] = build_module(**kw)
    return _MODULE_CACHE[key]


def kernel(x, wq, bq, wk, bk, wv, bv, w_lin, b_lin):
    from concourse.bass_utils import run_bass_kernel_spmd

    nc = _get_module()
    in_maps = make_core_inputs(x, wq, bq, wk, bk, wv, bv, w_lin, b_lin)
    res = run_bass_kernel_spmd(nc, in_maps, core_ids=list(range(N_CORES)))
    full = np.empty((1, HEADS * HID, H_IMG, OUT_DIM), np.float32)
    for h in range(HEADS):
        o = res.results[h]["out"].reshape(OUT_DIM, H_IMG, HID)
        full[0, HID * h : HID * (h + 1)] = o.transpose(2, 1, 0)
    return full


# revision 50
# speedup vs baseline: 1.6371x; 1.0073x over previous
"""Trainium2 Bass kernel for nn_MultiHeadSelfAttention2d (fp8 redesign).

Reference computation (B=1, C=64, H=32, W=128, HEADS=8, HIDDEN=16):
  q/k/v = 1x1 conv over channels (+bias), per-head attention over N=H*W=4096
  positions, softmax(q k^T / sqrt(16)), out = attn @ v, then a Linear over the
  W axis (W == HEADS*HIDDEN == 128) producing (1, 128, 32, 64).

Distribution: one head per NeuronCore -> 8 cores, fully independent.

Per-core dataflow:
  - proj:   V^T [m, d_aug] via X-stationary bf16 matmuls (bias + ones col
            folded into the augmented wva, all scaled x16), converted to
            fp8e4.  Q,K = W_aug x (bias row folded, x16) -> PSUM -> fp8e4
            in [d=16, (q|k), plane, n] layout where plane 1 is zeros.
  - S^T:    fp8 DoubleRow matmuls: lhsT = K[16,2,128] (plane1 = 0), rhs =
            Q[16,2,512] -> S^T*256 in PSUM at 0.5 cycles/col.
  - exp:    exp(u/1024) from PSUM -> fp8e4 P^T pair tiles [128, 2, n],
            split between ACT (hw exp) and DVE (cubic-poly custom op) by a
            greedy static schedule.
  - AV:     transposed accumulation: stationary = P^T pair [128,2,128],
            moving = V_aug pair [128,2,17] -> out [128(n=w), 17] per image
            row h, PSUM-accumulated over the 16 m-chunk pairs (DoubleRow).
            Output arrives already transposed; col 16 is the softmax
            denominator (V_aug ones column).
  - norm:   strided reciprocal of col 16, stride-0-broadcast tensor_tensor
            multiply -> OF [128(w), 32(h), 16(c)] bf16.
  - linear: out[o, (h,c)] = W_lin^T-stationary matmul over w + per-partition
            bias -> DMA out [64, 512] f32 (host transposes).
"""

import os
from contextlib import ExitStack

import ml_dtypes
import numpy as np

import concourse.bass as bass
import concourse.tile as tile
from concourse import bacc, mybir

# ---------------------------------------------------------------------------
# Problem constants (hardcoded per the task contract)
HEADS = 8
HID = 16
C_IN = 64
OUT_DIM = 64
H_IMG = 32
W_IMG = 128
N_TOK = H_IMG * W_IMG  # 4096
N_CORES = 8

BF16 = mybir.dt.bfloat16
F32 = mybir.dt.float32
FP8 = mybir.dt.float8e4
NPF8 = ml_dtypes.float8_e4m3

QK_SCALE = 16.0  # host-side scale on wq/wk/wv (and their biases)

# ---------------------------------------------------------------------------
# Custom DVE (vector engine) op: out = (((c3*u + c2)*u + c1)*u + 1)^2
# With c1=1/2048, c2=1/(2*2048^2), c3=1/(6*2048^3) this is exp(u/1024) to
# ~1e-5 rel for |u| < 220 (S*256 observed < 220).  Lets the Vector engine
# share softmax-exp work with the Scalar engine.
from concourse.dve_spec import Spec, Src0, C0, C1, C2, One, sq, lower
from concourse.dve_uop import DveOpSpec
from concourse import dve_ops
from concourse.dve_table_gen import dve_ver_for

EXP_C1 = 1.0 / 2048.0
EXP_C2 = 1.0 / (2.0 * 2048.0**2)
EXP_C3 = 1.0 / (6.0 * 2048.0**3)


def _exp_ref(in0, in1, c0, c1, c2):
    u = in0.astype(np.float32)
    q = ((np.float32(c2) * u + np.float32(c1)) * u + np.float32(c0)) * u + np.float32(
        1.0
    )
    return q * q


def _register_exp_op():
    name = "EXP_QTR_POLY_ANT"
    for op in dve_ops.OPS:
        if op.name == name:
            return op
    body = sq(((Src0 * C2 + C1) * Src0 + C0) * Src0 + One)
    spec = Spec(body=body, reference=_exp_ref)
    row = max(dve_ops._SUB_OPCODE_FOR_NAME.values()) + 1
    assert row < 0x20
    dve_ops._SUB_OPCODE_FOR_NAME[name] = row
    shas = {}
    for ver in ("v3", "v4"):
        try:
            uops = lower(spec, ver=ver)
            shas[ver] = DveOpSpec(name=name, opcode=row, uops=uops, rd1_en=False).sha(
                ver
            )
        except Exception:
            pass
    op = dve_ops.DveOp(name, spec, subdim=False, uops_sha=shas)
    dve_ops.OPS.append(op)
    dve_ops.CUSTOM_DVE_SPECS[name] = spec
    return op


EXP_OP = _register_exp_op()

# expm1 variant: out = ((c2*u + c1)*u + c0)*u  ~=  exp(u/1024) - 1
# (delta form keeps fp8 quantization error ~4x smaller near P=1)
EXPM1_C0 = 1.0 / 1024.0
EXPM1_C1 = 1.0 / 2097152.0
EXPM1_C2 = 4.0 / (3.0 * 2048.0**3)


def _expm1_ref(in0, in1, c0, c1, c2):
    u = in0.astype(np.float32)
    return ((np.float32(c2) * u + np.float32(c1)) * u + np.float32(c0)) * u


def _register_expm1_op():
    name = "EXPM1_CUBIC_ANT"
    for op in dve_ops.OPS:
        if op.name == name:
            return op
    body = ((Src0 * C2 + C1) * Src0 + C0) * Src0
    spec = Spec(body=body, reference=_expm1_ref)
    row = max(dve_ops._SUB_OPCODE_FOR_NAME.values()) + 1
    assert row < 0x20
    dve_ops._SUB_OPCODE_FOR_NAME[name] = row
    shas = {}
    for ver in ("v3", "v4"):
        try:
            uops = lower(spec, ver=ver)
            shas[ver] = DveOpSpec(name=name, opcode=row, uops=uops, rd1_en=False).sha(
                ver
            )
        except Exception:
            pass
    op = dve_ops.DveOp(name, spec, subdim=False, uops_sha=shas)
    dve_ops.OPS.append(op)
    dve_ops.CUSTOM_DVE_SPECS[name] = spec
    return op


EXPM1_OP = _register_expm1_op()

# m-chunks whose exp runs on DVE as delta=expm1 (odd chunks minus {1,3}):
# 14 of 32, balancing ACT 18*4*1038ns vs DVE 14*8*658ns.
D_CHUNKS = frozenset(mc for mc in range(1, 32, 2)) - {1, 3}


def _chunk_splits(n):
    """Split n columns into s-tile widths of 1024 (PSUM: 2 banks/tile)."""
    assert n % 1024 == 0
    return [(o, 1024) for o in range(0, n, 1024)]


# ---------------------------------------------------------------------------
def build_module(n_tok: int = N_TOK, act_bias_ns: float = -1100.0, pt_bufs: int = 5, pend_max: int = 32, tail0_at: int = 20):
    """Builds (and bacc-compiles) the per-core Bass module."""
    assert n_tok % 1024 == 0
    mch = n_tok // 128  # m chunks (32)
    npair = mch // 2  # DoubleRow pairs (16)
    nh = n_tok // W_IMG  # image rows (32)
    f_tot = nh * HID  # (h,c) columns of the final output (512)

    nc = bacc.Bacc()

    xin = nc.dram_tensor("xin", [C_IN + 1, n_tok], BF16, kind="ExternalInput")
    wpk = nc.dram_tensor("wpk", [C_IN + 1, 2 * HID + HID + 1], BF16, kind="ExternalInput")
    ztil = nc.dram_tensor("ztil_v7", [HID, n_tok], FP8, kind="ExternalInput")
    wlt = nc.dram_tensor("wlt", [W_IMG, OUT_DIM], BF16, kind="ExternalInput")
    blb = nc.dram_tensor("blb", [OUT_DIM, 1], F32, kind="ExternalInput")
    out = nc.dram_tensor("out", [OUT_DIM, f_tot], F32, kind="ExternalOutput")

    # --- engine assignment: time-balanced greedy over per-engine rings -----
    ACT_RATE, ACT_INIT = 1.0 / 1.2, 185.0
    DVE_RATE, DVE_INIT = 1.0 / 0.96, 125.0
    eng_t = {"A": 1283.0 + act_bias_ns, "D": 0.0}

    def charge_engine(eng, width):
        if eng == "A":
            eng_t["A"] += width * ACT_RATE + ACT_INIT
        else:
            eng_t["D"] += width * DVE_RATE + DVE_INIT

    def pick_engine(width_a, width_d=None):
        """Pick engine for an op of width_a on ACT / width_d on DVE."""
        if width_d is None:
            width_d = width_a
        ca = width_a * ACT_RATE + ACT_INIT
        cd = width_d * DVE_RATE + DVE_INIT
        if eng_t["A"] + ca <= eng_t["D"] + cd:
            eng_t["A"] += ca
            return "A"
        eng_t["D"] += cd
        return "D"

    with tile.TileContext(nc) as tc, ExitStack() as ctx:
        const = ctx.enter_context(tc.tile_pool(name="const", bufs=1))

        WPK = const.tile([C_IN + 1, 2 * HID + HID + 1], BF16)
        nc.sync.dma_start(WPK[:], wpk.ap())
        WQA = WPK[:, 0:HID]
        WKA = WPK[:, HID : 2 * HID]
        WVA = WPK[:, 2 * HID : 3 * HID + 1]

        XB = const.tile([C_IN + 1, n_tok], BF16)
        QK8 = const.tile([HID, 2, 2, n_tok], FP8)
        WL = const.tile([W_IMG, OUT_DIM], BF16)
        BLB = const.tile([OUT_DIM, 1], F32)
        for q0, q1 in [(0, 512), (512, 1024), (1024, 2048), (2048, 3072), (3072, 4096)]:
            q0 = q0 * n_tok // 4096
            q1 = q1 * n_tok // 4096
            nc.sync.dma_start(XB[:, q0:q1], xin.ap()[:, q0:q1])
        nc.sync.dma_start(QK8[:, 0, 1, :], ztil.ap())
        nc.sync.dma_start(QK8[:, 1, 1, :], ztil.ap())
        nc.sync.dma_start(WL[:], wlt.ap())
        nc.sync.dma_start(BLB[:], blb.ap())

        VA = const.tile([128, npair, 2, HID + 1], FP8)
        VASD = const.tile([128, 4, HID + 1], F32)  # per-group D-chunk V sums
        VSD = const.tile([128, HID + 1], F32)  # sum over all D-chunk m of V_aug
        VSDH = const.tile([128, HID + 1], BF16)
        VSDL = const.tile([128, HID + 1], BF16)
        ONESB = const.tile([128, 128], BF16)
        RC = const.tile([128, nh], F32)
        OF = const.tile([128, nh, HID], BF16)
        RES = const.tile([OUT_DIM, f_tot], F32)

        # ---- attention: S^T (fp8 DoubleRow) -> exp -> AV (transposed) ------
        with tc.tile_pool(name="s_pool", bufs=1, space="PSUM") as s_pool, \
             tc.tile_pool(name="av_pool", bufs=1, space="PSUM") as av_pool, \
             tc.tile_pool(name="pt_pool", bufs=pt_bufs) as pt_pool:
            # one [128, 17] accumulator per image row h, packed 16 per bank
            av = av_pool.tile([128, nh, 32], F32, tag="av", name="av")

            # ---- projections, emitted through the same s-tile rings --------
            for sc in range(n_tok // 512):
                ps = s_pool.tile([128, 1024], F32, tag="sa", bufs=2, name="sa")
                cs = slice(512 * sc, 512 * sc + 512)
                nc.tensor.matmul(ps[0:HID, 0:512], lhsT=WQA, rhs=XB[:, cs])
                nc.tensor.matmul(ps[0:HID, 512:1024], lhsT=WKA, rhs=XB[:, cs])
                src_ap = ps[0:HID, :].rearrange("p (a b) -> p a b", b=512)
                dst = QK8[:, :, 0, cs]
                if pick_engine(1024) == "A":
                    nc.scalar.activation(
                        dst, src_ap, mybir.ActivationFunctionType.Copy
                    )
                else:
                    nc.vector.tensor_copy(dst, src_ap)
                if sc % 2 == 1:
                    g = sc // 2
                    psv = s_pool.tile([128, 512], F32, tag="sd", bufs=2, name="sd")
                    for vj in range(8):
                        mc = 8 * g + vj
                        nc.tensor.matmul(
                            psv[:, 64 * vj : 64 * vj + HID + 1],
                            lhsT=XB[:, 128 * mc : 128 * mc + 128],
                            rhs=WVA,
                        )
                    vsrc = psv[:].rearrange("p (a b) -> p a b", b=64)[:, :, 0 : HID + 1]
                    vdst = VA[:, 4 * g : 4 * g + 4, :, :]
                    if pick_engine(136) == "A":
                        nc.scalar.activation(
                            vdst, vsrc, mybir.ActivationFunctionType.Copy
                        )
                    else:
                        nc.vector.tensor_copy(vdst, vsrc)
                    # f32 sums of this group's D-chunk V columns (delta
                    # add-back); D positions: g0 -> {5,7}, else odd
                    vt = psv[:].rearrange("p (a b) -> p b a", b=64)
                    dsl = (
                        vt[:, 0 : HID + 1, 5:8:2]
                        if g == 0
                        else vt[:, 0 : HID + 1, 1:8:2]
                    )
                    nc.vector.tensor_reduce(
                        VASD[:, g, :], dsl, axis=mybir.AxisListType.X,
                        op=mybir.AluOpType.add,
                    )
            nc.vector.tensor_reduce(
                VSD[:], VASD[:].rearrange("p g d -> p d g"),
                axis=mybir.AxisListType.X, op=mybir.AluOpType.add,
            )
            nc.vector.tensor_copy(VSDH[:], VSD[:])
            nc.vector.tensor_tensor(
                VSDL[:], VSD[:], VSDH[:], op=mybir.AluOpType.subtract
            )
            nc.gpsimd.memset(ONESB[:], 1.0)

            def av_mm(j, pt, h):
                nc.tensor.matmul(
                    av[:, h, 0 : HID + 1],
                    lhsT=pt[:, :, 128 * h : 128 * h + 128],
                    rhs=VA[:, j, :, :],
                    perf_mode=mybir.MatmulPerfMode.DoubleRow,
                    start=False,
                    stop=(j == npair - 1),
                    skip_group_check=True,
                )

            # seed each accumulator with sum_{m in D-chunks} V_aug[m, :]
            # (bf16 hi+lo ones-matmuls; delta-form chunks contribute P-1)
            for h in range(nh):
                # start=True only on the first write of each PSUM bank: a
                # start re-zeroes the whole bank's accumulation group, so
                # later regions must join with start=False
                nc.tensor.matmul(
                    av[:, h, 0 : HID + 1],
                    lhsT=ONESB[:],
                    rhs=VSDH[:],
                    start=(h % 16 == 0),
                    stop=False,
                    skip_group_check=True,
                )
                nc.tensor.matmul(
                    av[:, h, 0 : HID + 1],
                    lhsT=ONESB[:],
                    rhs=VSDL[:],
                    start=False,
                    stop=False,
                    skip_group_check=True,
                )

            def emit_slot(j, t, col, w, eng, pt):
                mc = 2 * j + t
                if eng == "A":
                    s = s_pool.tile([128, 1024], F32, tag="sa", bufs=2, name="sa")
                else:
                    s = s_pool.tile([128, 512], F32, tag="sd", bufs=2, name="sd")
                for sub in range(0, w, 512):
                    nc.tensor.matmul(
                        s[:, sub : sub + 512],
                        lhsT=QK8[:, 1, :, 128 * mc : 128 * mc + 128],
                        rhs=QK8[:, 0, :, col + sub : col + sub + 512],
                        perf_mode=mybir.MatmulPerfMode.DoubleRow,
                    )
                dstp = pt[:, t, col : col + w]
                if eng == "A":
                    nc.scalar.activation(
                        dstp,
                        s[:, 0:w],
                        mybir.ActivationFunctionType.Exp,
                        scale=1.0 / 1024.0,
                    )
                else:
                    nc.vector._custom_dve(
                        EXPM1_OP,
                        out=dstp,
                        in0=s[:, 0:w],
                        s0=EXPM1_C0,
                        s1=EXPM1_C1,
                        imm2=EXPM1_C2,
                    )

            # build per-engine slot streams (chunk -> engine fixed by
            # D_CHUNKS), then merge by projected engine finish time so the
            # PE feeds both rings concurrently
            streams = {"A": [], "D": []}
            slots_left = [0] * npair
            for j in range(npair):
                for t in range(2):
                    mc = 2 * j + t
                    eng = "D" if mc in D_CHUNKS else "A"
                    w = 1024 if eng == "A" else 512
                    for col in range(0, n_tok, w):
                        streams[eng].append((j, t, col, w))
                        slots_left[j] += 1

            pt_tiles = {}

            def get_pt(j):
                if j not in pt_tiles:
                    pt_tiles[j] = pt_pool.tile(
                        [128, 2, n_tok], FP8, tag="pt", name="pt"
                    )
                return pt_tiles[j]

            def bank_tail(b):
                hs = slice(16 * b, 16 * b + 16)
                nc.vector.reciprocal(RC[:, hs], av[:, hs, HID : HID + 1])
                nc.vector.tensor_tensor(
                    OF[:, hs, :],
                    av[:, hs, 0:HID],
                    RC[:, hs].unsqueeze(2).broadcast_to([128, 16, HID]),
                    op=mybir.AluOpType.mult,
                )
                fs = slice(256 * b, 256 * b + 256)
                psf = s_pool.tile([128, 512], F32, tag="sd", bufs=2, name="sd")
                nc.tensor.matmul(
                    psf[0:OUT_DIM, 0:256],
                    lhsT=WL[:],
                    rhs=OF[:, 16 * b : 16 * b + 16, :],
                )
                nc.scalar.activation(
                    RES[:, fs], psf[0:OUT_DIM, 0:256],
                    mybir.ActivationFunctionType.Identity, bias=BLB[:],
                )
                nc.sync.dma_start(out.ap()[:, fs], RES[:, fs])

            pend = []
            ii = {"A": 0, "D": 0}
            lastj = npair - 1
            prog = [0, 0]  # column progress of the last pair's two planes
            next_h = 0  # next last-pair AV row to emit
            while ii["A"] < len(streams["A"]) or ii["D"] < len(streams["D"]):
                if ii["A"] >= len(streams["A"]):
                    eng = "D"
                elif ii["D"] >= len(streams["D"]):
                    eng = "A"
                else:
                    wa = streams["A"][ii["A"]][3]
                    wd = streams["D"][ii["D"]][3]
                    ca = eng_t["A"] + wa * ACT_RATE + ACT_INIT
                    cd = eng_t["D"] + wd * DVE_RATE + DVE_INIT
                    eng = "A" if ca <= cd else "D"
                j, t, col, w = streams[eng][ii[eng]]
                ii[eng] += 1
                charge_engine(eng, w)
                emit_slot(j, t, col, w, eng, get_pt(j))
                slots_left[j] -= 1
                if slots_left[j] == 0 and j != lastj:
                    pend.extend((j, pt_tiles[j], h) for h in range(nh))
                if j == lastj:
                    # progressive tail: drain earlier pairs, then emit each
                    # last-pair AV stop as soon as both planes cover its row
                    for args in pend:
                        av_mm(*args)
                    pend = []
                    prog[t] = col + w
                    lim = min(prog) // W_IMG
                    while next_h < lim:
                        av_mm(lastj, get_pt(lastj), next_h)
                        next_h += 1
                        if next_h == tail0_at:
                            bank_tail(0)
                else:
                    while len(pend) > pend_max:
                        av_mm(*pend.pop(0))
            while next_h < nh:
                av_mm(lastj, get_pt(lastj), next_h)
                next_h += 1
                if next_h == tail0_at:
                    bank_tail(0)
            for args in pend:
                av_mm(*args)
            bank_tail(1)

    nc.compile()
    return nc


# ---------------------------------------------------------------------------
def make_core_inputs(x, wq, bq, wk, bk, wv, bv, w_lin, b_lin, n_tok=N_TOK):
    """Host-side prep: full inputs -> list of 8 per-core input dicts."""
    X = np.asarray(x, np.float32).reshape(C_IN, -1)[:, :n_tok]
    xa = np.ones((C_IN + 1, n_tok), np.float32)
    xa[:C_IN] = X
    xin = xa.astype(ml_dtypes.bfloat16)
    wlt = np.ascontiguousarray(np.asarray(w_lin, np.float32).T).astype(
        ml_dtypes.bfloat16
    )
    blb = np.asarray(b_lin, np.float32).reshape(OUT_DIM, 1)
    ztil = np.zeros((HID, n_tok), NPF8)

    s = QK_SCALE
    maps = []
    for h in range(HEADS):
        sl = slice(HID * h, HID * (h + 1))

        def aug(w, b, ones_col=False):
            d = HID + 1 if ones_col else HID
            m = np.zeros((C_IN + 1, d), np.float32)
            m[0:C_IN, 0:HID] = s * np.asarray(w, np.float32)[sl].T
            m[C_IN, 0:HID] = s * np.asarray(b, np.float32)[sl]
            if ones_col:
                m[C_IN, HID] = s
            return m.astype(ml_dtypes.bfloat16)

        wpk = np.concatenate(
            [
                np.asarray(aug(wq, bq), np.float32),
                np.asarray(aug(wk, bk), np.float32),
                np.asarray(aug(wv, bv, ones_col=True), np.float32),
            ],
            axis=1,
        ).astype(ml_dtypes.bfloat16)
        maps.append(
            {
                "xin": xin,
                "wpk": wpk,
                "ztil_v7": ztil,
                "wlt": wlt,
                "blb": blb,
            }
        )
    return maps


_MODULE_CACHE = {}


def _get_module(**kw):
    key = tuple(sorted(kw.items()))
    if key not in _MODULE_CACHE:
        _MODULE_CACHE[key] = build_module(**kw)
    return _MODULE_CACHE[key]


def kernel(x, wq, bq, wk, bk, wv, bv, w_lin, b_lin):
    from concourse.bass_utils import run_bass_kernel_spmd

    nc = _get_module()
    in_maps = make_core_inputs(x, wq, bq, wk, bk, wv, bv, w_lin, b_lin)
    res = run_bass_kernel_spmd(nc, in_maps, core_ids=list(range(N_CORES)))
    full = np.empty((1, HEADS * HID, H_IMG, OUT_DIM), np.float32)
    for h in range(HEADS):
        o = res.results[h]["out"].reshape(OUT_DIM, H_IMG, HID)
        full[0, HID * h : HID * (h + 1)] = o.transpose(2, 1, 0)
    return full
